# revision 55
# baseline (speedup 1.0000x reference)
"""Bass/Trainium2 kernel for nn_EquivariantPosUpdate — 8-core edge-parallel.

v2: transposed depthwise-TP. Per core, 1024 edges in 2 dtp-tiles of DT=512
(4 subtiles of 128 for gather/RBF/radial/adaLN work that needs edges on
partitions).

Key ideas vs v1:
  - bf16 everywhere PE touches (4x matmul, 2x DVE) with scales folded on host
  - TP weights computed TRANSPOSED: w^T[(u,v), e] = matmul(W3-block, h2T),
    so the v-contraction becomes a one-hot PE matmul (PSUM row writes)
    instead of a DVE TensorReduce — the v1 bottleneck (238us of DVE reduces)
    moves to the 4x-faster, underutilized PE
  - all epilogues run in [feat, e] layout; nt/et linears consume them
    directly as lhsT (no fs/fv transposes)
  - one activation table (exp/ln/square/copy): rstd = exp(-0.5*ln(var)),
    silu via exp + fast-reciprocal on DVE
"""
import sys
sys.path.insert(0, '/opt/trn_rl_repo')
import numpy as np
from contextlib import ExitStack

import concourse.bass as bass
import concourse.bacc as bacc
import concourse.mybir as mybir
import concourse.tile as tile
from concourse.bass import AP, IndirectOffsetOnAxis
from concourse.masks import make_identity

F32 = mybir.dt.float32
BF16 = mybir.dt.bfloat16
I32 = mybir.dt.int32
AX = mybir.AxisListType
OP = mybir.AluOpType
ACTF = mybir.ActivationFunctionType

N, E, G, NB = 2048, 8192, 64, 128
NC_CORES = 8
EC = E // NC_CORES          # 1024
P = 128
DT = 512                    # dtp tile (edges on free axis)
NDT = EC // DT              # 2
NSUB = DT // P              # 4
M0, M1 = 64, 32
S_TP = 96
CUTOFF = 5.0
NCHUNK = N // P             # 16

ROWS = {}
_off = 0
for _n, _w in [('nf_g1', 64), ('nf_b1', 64), ('nf_g2', 64), ('nf_b2', 64),
               ('ef_g1', 64), ('ef_b1', 64), ('ef_g2', 64), ('ef_b2', 64),
               ('src_bs', 64), ('dst_bs', 64),
               ('sp_b1', 32), ('spW2r', 32),
               ('sp_b2', 1), ('eps', 1), ('normbt', 192)]:
    ROWS[_n] = (_off, _w)
    _off += _w
RWID = _off


def rows_slice(rep, name):
    off, w = ROWS[name]
    return rep[:, off:off + w]


def ap3(t, dims, offset=0):
    base = t[:, :] if not isinstance(t, AP) else t
    ap = AP(base.tensor, base.offset + offset, [base.ap[0]] + [list(d) for d in dims])
    return ap


def build_nc():
    nc = bacc.Bacc("TRN2", target_bir_lowering=False, debug=False,
                   num_devices=NC_CORES)
    T = {}

    def din(name, shape, dtype=F32):
        T[name] = nc.dram_tensor(name, shape, dtype, kind="ExternalInput")
        return T[name]

    din('hn_T', [320, N], BF16); din('he_T', [160, EC], BF16)
    din('dist_r', [1, EC])
    din('edgef', [EC, 5]); din('edgei', [EC, 3], I32)
    din('t_T', [128, G], BF16); din('normWt', [128, 2 * S_TP], BF16)
    din('rows', [1, RWID])
    din('rbf_A', [NB, 1]); din('rbf_B', [NB, 1])
    din('W1p', [NB, 128], BF16)
    din('nf_W2', [64, 64], BF16); din('ef_W2', [64, 64], BF16)
    din('W3nf', [64, 10240], BF16); din('W3ef', [64, 5120], BF16)
    din('Wsd', [128, 128], BF16); din('Wvsd', [64, 64], BF16)
    din('nt_Ws', [S_TP, 64], BF16); din('nt_Wv', [128, 32], BF16)
    din('et_Ws', [64, 64], BF16); din('et_Wv', [32, 32], BF16)
    din('sp_W1', [S_TP, 32], BF16)
    din('UBss', [128, 2048], BF16); din('UBsv', [128, 1024], BF16)
    din('UB4', [128, 256], BF16)
    din('U3', [96, 32], BF16); din('R96', [96, 384], BF16)
    din('bcols', [96, 4])
    out = nc.dram_tensor('out', [128, NCHUNK * 3], F32, kind="ExternalOutput")
    T['out'] = out
    T['proj_src'] = nc.dram_tensor('proj_src', [N, 160], BF16)
    T['proj_dst'] = nc.dram_tensor('proj_dst', [N, 160], BF16)
    T['mod_d'] = nc.dram_tensor('mod_d', [G, 2 * S_TP], F32)

    with tile.TileContext(nc) as tc:
        with ExitStack() as ctx:
            _build(ctx, tc, nc, T)
    nc.compile()
    return nc


def _build(ctx, tc, nc, T):
    consts = ctx.enter_context(tc.tile_pool(name="consts", bufs=1))
    setup = ctx.enter_context(tc.tile_pool(name="setup", bufs=2))
    sb = ctx.enter_context(tc.tile_pool(name="sb", bufs=3))
    sbq = ctx.enter_context(tc.tile_pool(name="sbq", bufs=3))
    sbQ = ctx.enter_context(tc.tile_pool(name="sbQ", bufs=6))
    sbg = ctx.enter_context(tc.tile_pool(name="sbg", bufs=2))
    sbT = ctx.enter_context(tc.tile_pool(name="sbT", bufs=2))
    sbc = ctx.enter_context(tc.tile_pool(name="sbc", bufs=8))
    sbx = ctx.enter_context(tc.tile_pool(name="sbx", bufs=4))
    ps = ctx.enter_context(tc.tile_pool(name="ps", bufs=2, space="PSUM"))
    psm = ctx.enter_context(tc.tile_pool(name="psm", bufs=1, space="PSUM"))
    psb = ctx.enter_context(tc.tile_pool(name="psb", bufs=1, space="PSUM"))
    psw = ctx.enter_context(tc.tile_pool(name="psw", bufs=2, space="PSUM"))
    psp = ctx.enter_context(tc.tile_pool(name="psp", bufs=2, space="PSUM"))
    dma = nc.sync.dma_start

    def load(name, pool=consts):
        t = pool.tile(T[name].shape, T[name].dtype, tag="ld_" + name,
                      name="ld_" + name)
        dma(t[:], T[name][:])
        return t

    # ---------------- setup ----------------
    # pin one act table (exp/ln/square/copy) so the table-load pass never
    # needs to swap between exp-only and ln-only sets
    PIN_ACT_TABLE = True
    if PIN_ACT_TABLE:
        from concourse.hw_specs import get_activation_tables
        _tabs = list(get_activation_tables(nc.m.arch))
        _tid = _tabs.index('natural_log_exp_and_others')
        nc.scalar.add_instruction(mybir.InstLoadActFuncSet(
            name=nc.get_next_instruction_name(), ins=[], outs=[],
            act_func_set_id=_tid))
    ident = consts.tile([P, P], F32)
    make_identity(nc, ident[:])
    identB = consts.tile([P, P], BF16)
    nc.vector.tensor_copy(identB[:], ident[:])
    iota_i = consts.tile([P, P], I32)
    nc.gpsimd.iota(iota_i[:], pattern=[[1, P]], base=0, channel_multiplier=0)
    iota_f = consts.tile([P, P], F32)
    nc.vector.tensor_copy(iota_f[:], iota_i[:])

    rows1 = consts.tile([1, RWID], F32)
    dma(rows1[:], T['rows'][:])
    o_nbt = ROWS['normbt'][0]
    nc.vector.tensor_scalar_add(rows1[:, o_nbt + S_TP:o_nbt + 2 * S_TP],
                                rows1[:, o_nbt + S_TP:o_nbt + 2 * S_TP], 1.0)
    rep = consts.tile([P, RWID], F32)
    nc.gpsimd.partition_broadcast(rep[:], rows1[:])

    Wsd = load('Wsd'); Wvsd = load('Wvsd')
    WvsdHi = consts.tile([128, 64], BF16, tag="wvhi")
    nc.vector.tensor_copy(WvsdHi[64:128, :], Wvsd[:, :])
    rbf_A = load('rbf_A'); rbf_B = load('rbf_B')
    dist_r = load('dist_r')
    W1p = load('W1p')
    W2nf = load('nf_W2'); W2ef = load('ef_W2')
    ntWs = load('nt_Ws'); ntWv = load('nt_Wv')
    etWs = load('et_Ws'); etWv = load('et_Wv')
    spW1 = load('sp_W1')
    normWt = load('normWt'); tT = load('t_T')
    U3 = load('U3'); R96 = load('R96')
    bcols = load('bcols')
    UBss = load('UBss'); UBsv = load('UBsv'); UB4 = load('UB4')
    W3nf = consts.tile(T['W3nf'].shape, BF16, tag="ld_W3nf", name="ld_W3nf")
    nc.scalar.dma_start(W3nf[:], T['W3nf'][:])
    W3ef = consts.tile(T['W3ef'].shape, BF16, tag="ld_W3ef", name="ld_W3ef")
    nc.scalar.dma_start(W3ef[:], T['W3ef'][:])

    _uid = [0]

    def uid(tag):
        _uid[0] += 1
        return f"{tag}_{_uid[0]}"

    def silu(dst, src_ap, width, pool):
        e_ = pool.tile([P, width], F32, tag="silu_e", name=uid("se"))
        nc.scalar.activation(e_[:], src_ap, ACTF.Exp, scale=-1.0)
        d_ = pool.tile([P, width], F32, tag="silu_d", name=uid("sd"))
        nc.vector.tensor_scalar_add(d_[:], e_[:], 1.0)
        r_ = pool.tile([P, width], F32, tag="silu_r", name=uid("sr"))
        nc.vector.reciprocal_approx_fast(out=r_[:], in_=d_[:])
        nc.vector.tensor_tensor(dst, src_ap, r_[:], op=OP.mult)

    def rstd_from_var(var_ap, inv_width, pool):
        lnv = pool.tile([P, 1], F32, tag="lnv", name=uid("lnv"))
        nc.scalar.activation(lnv[:], var_ap, ACTF.Ln, scale=inv_width,
                             bias=rep[:, ROWS['eps'][0]:ROWS['eps'][0] + 1])
        rstd = pool.tile([P, 1], F32, tag="rstd", name=uid("rstd"))
        nc.scalar.activation(rstd[:], lnv[:], ACTF.Exp, scale=-0.5)
        return rstd

    # ---------------- phase A: node projections (bf16) ----------------
    # hn_T host row order: [s(128) | x0(64) | x1(64) | x2(64)] so each load
    # is contiguous. proj writes packed per chunk, issued on the Act queue.
    for c in range(NCHUNK):
        ldq = dma
        stq = nc.scalar.dma_start
        hsT = setup.tile([128, P], BF16, tag="hsT")
        ldq(hsT[:], T['hn_T'][0:128, c * P:(c + 1) * P])
        psrc = setup.tile([P, 160], BF16, tag="psrc")
        pdst = setup.tile([P, 160], BF16, tag="pdst")
        pp = ps.tile([P, 128], F32, tag="ps_small")
        nc.tensor.matmul(pp[:], hsT[:], Wsd[:], start=True, stop=True)
        nc.vector.tensor_tensor(
            psrc[:, 0:64], pp[:, 0:64],
            rep[:, ROWS['src_bs'][0]:ROWS['src_bs'][0] + 64], op=OP.add)
        nc.vector.tensor_tensor(
            pdst[:, 0:64], pp[:, 64:128],
            rep[:, ROWS['dst_bs'][0]:ROWS['dst_bs'][0] + 64], op=OP.add)
        hv01 = setup.tile([128, P], BF16, tag="hv01")
        ldq(hv01[:], T['hn_T'][128:256, c * P:(c + 1) * P])
        hv2 = setup.tile([64, P], BF16, tag="hv2")
        ldq(hv2[:], T['hn_T'][256:320, c * P:(c + 1) * P])
        for x in range(3):
            if x == 0:
                lhs, rhs = hv01[0:64, :], Wvsd[:, :]
            elif x == 1:
                lhs, rhs = hv01[64:128, :], WvsdHi[64:128, :]
            else:
                lhs, rhs = hv2[:, :], Wvsd[:, :]
            pv = ps.tile([P, 64], F32, tag="ps_small")
            nc.tensor.matmul(pv[:], lhs, rhs, start=True, stop=True)
            nc.scalar.copy(psrc[:, 64 + 32 * x:96 + 32 * x], pv[:, 0:32])
            nc.scalar.copy(pdst[:, 64 + 32 * x:96 + 32 * x], pv[:, 32:64])
        stq(T['proj_src'][c * P:(c + 1) * P, :], psrc[:])
        stq(T['proj_dst'][c * P:(c + 1) * P, :], pdst[:])

    pm = ps.tile([G, 2 * S_TP], F32, tag="ps_small")
    nc.tensor.matmul(pm[:], tT[:], normWt[:], start=True, stop=True)
    msb = setup.tile([G, 2 * S_TP], F32)
    nc.vector.tensor_tensor(
        msb[:], pm[:], rep[0:G, o_nbt:o_nbt + 2 * S_TP], op=OP.add)
    dma(T['mod_d'][:], msb[:])

    # ---------------- phase B ----------------
    def ln_silu_stage(src_aps, keys, width, pool):
        """Stage-major LayerNorm+SiLU over n independent chains.
        src_aps: list of [P,width] APs; keys: (gname, bname) per chain.
        Returns list of bf16 [P,width] outputs."""
        n = len(src_aps)
        mus = [pool.tile([P, 1], F32, tag="lnmu", name=uid("mu")) for _ in range(n)]
        for i, a in enumerate(src_aps):
            nc.vector.tensor_reduce(mus[i][:], a, axis=AX.X, op=OP.add)
        for i in range(n):
            nc.vector.tensor_scalar_mul(mus[i][:], mus[i][:], 1.0 / width)
        cens = [pool.tile([P, width], F32, tag="lncen", name=uid("cen"))
                for _ in range(n)]
        for i, a in enumerate(src_aps):
            nc.vector.tensor_scalar(cens[i][:], a, mus[i][:, :1], None,
                                    op0=OP.subtract)
        vars_ = [pool.tile([P, 1], F32, tag="lnvar", name=uid("var"))
                 for _ in range(n)]
        for i in range(n):
            sqv = sb.tile([P, width], F32, tag="lnsq", name=uid("sq"))
            nc.scalar.activation(sqv[:], cens[i][:], ACTF.Square,
                                 accum_out=vars_[i][:])
        lnvs = [pool.tile([P, 1], F32, tag="lnv", name=uid("lnv"))
                for _ in range(n)]
        for i in range(n):
            nc.scalar.activation(lnvs[i][:], vars_[i][:], ACTF.Ln,
                                 scale=1.0 / width,
                                 bias=rep[:, ROWS['eps'][0]:ROWS['eps'][0] + 1])
        rstds = [pool.tile([P, 1], F32, tag="rstd", name=uid("rstd"))
                 for _ in range(n)]
        for i in range(n):
            nc.scalar.activation(rstds[i][:], lnvs[i][:], ACTF.Exp, scale=-0.5)
        dests = [pool.tile([P, width], F32, tag="lndst", name=uid("dst"))
                 for _ in range(n)]
        for i in range(n):
            nc.vector.scalar_tensor_tensor(dests[i][:], cens[i][:],
                                           rstds[i][:, :1],
                                           rows_slice(rep, keys[i][0]),
                                           op0=OP.mult, op1=OP.mult)
        for i in range(n):
            nc.vector.tensor_tensor(dests[i][:], dests[i][:],
                                    rows_slice(rep, keys[i][1]), op=OP.add)
        # silu
        es_ = [pool.tile([P, width], F32, tag="silu_e", name=uid("se"))
               for _ in range(n)]
        for i in range(n):
            nc.scalar.activation(es_[i][:], dests[i][:], ACTF.Exp, scale=-1.0)
        for i in range(n):
            nc.vector.tensor_scalar_add(es_[i][:], es_[i][:], 1.0)
        rs_ = [pool.tile([P, width], F32, tag="silu_r", name=uid("sr"))
               for _ in range(n)]
        for i in range(n):
            nc.vector.reciprocal(rs_[i][:], es_[i][:])
        outs = [pool.tile([P, width], BF16, tag="lnout", name=uid("lo"))
                for _ in range(n)]
        for i in range(n):
            nc.vector.tensor_tensor(outs[i][:], dests[i][:], rs_[i][:], op=OP.mult)
        return outs

    g_modA, d_colsA, rvsA, srcfsA, asTs = [], [], [], [], []

    for td in range(NDT):
        E0 = td * DT
        tg = f"t{td}"

        # big per-512 tiles
        s1T = sbT.tile([64, DT], BF16, tag="s1T", name=f"s1T_{td}")
        v1T = sbT.tile([96, DT], BF16, tag="v1T", name=f"v1T_{td}")
        s2T = sbT.tile([64, DT], BF16, tag="s2T", name=f"s2T_{td}")
        v2T = sbT.tile([96, DT], BF16, tag="v2T", name=f"v2T_{td}")
        h2T = {p: sbT.tile([64, DT], BF16, tag=f"h2T{p}", name=f"h2T{p}_{td}")
               for p in ('nf', 'ef')}
        heT = sbT.tile([64, DT], BF16, tag="heT", name=f"heT_{td}")
        dma(heT[:], T['he_T'][0:64, E0:E0 + DT])
        hevT = [sbT.tile([32, DT], BF16, tag=f"hevT{x}", name=f"hevT{x}_{td}")
                for x in range(3)]
        for x in range(3):
            dma(hevT[x][:], T['he_T'][64 + 32 * x:96 + 32 * x, E0:E0 + DT])

        g_mod = []; d_cols = []; rvs = []; srcfs = []; x1sb = []

        # ---- subtile prep: gathers, transposes, RBF, radial ----
        for s in range(NSUB):
            e0 = E0 + s * P
            ef = sbg.tile([P, 5], F32, tag=f"ef{s}", name=uid("ef"))
            dma(ef[:], T['edgef'][e0:e0 + P, :])
            ei = sb.tile([P, 3], I32, tag="ei", name=uid("ei"))
            dma(ei[:], T['edgei'][e0:e0 + P, :])
            d_cols.append(ef[:, 0:1]); rvs.append(ef[:, 1:4]); srcfs.append(ef[:, 4:5])

            g_src = sb.tile([P, 160], BF16, tag="gsrc", name=uid("gs"))
            nc.gpsimd.indirect_dma_start(
                out=g_src[:], out_offset=None, in_=T['proj_src'][:],
                in_offset=IndirectOffsetOnAxis(ap=ei[:, 0:1], axis=0))
            g_dst = sb.tile([P, 160], BF16, tag="gdst", name=uid("gd"))
            nc.gpsimd.indirect_dma_start(
                out=g_dst[:], out_offset=None, in_=T['proj_dst'][:],
                in_offset=IndirectOffsetOnAxis(ap=ei[:, 1:2], axis=0))
            gm = sbg.tile([P, 2 * S_TP], F32, tag=f"gmod{s}", name=uid("gm"))
            nc.gpsimd.indirect_dma_start(
                out=gm[:], out_offset=None, in_=T['mod_d'][:],
                in_offset=IndirectOffsetOnAxis(ap=ei[:, 2:3], axis=0))
            g_mod.append(gm)

            co = s * P
            for (big, src_t, w) in ((s1T, g_src, 64), (v1T, g_src, 96),
                                    (s2T, g_dst, 64), (v2T, g_dst, 96)):
                off = 0 if w == 64 else 64
                tp = psb.tile([w, P], BF16, tag="ps_bf", name=uid("tp"))
                nc.tensor.transpose(tp[:], src_t[:, off:off + w], identB[:])
                nc.scalar.copy(big[:, co:co + P], tp[:])

            # RBF (transposed)
            d_rep = sb.tile([P, P], F32, tag="drep", name=uid("dr"))
            nc.gpsimd.partition_broadcast(d_rep[:], dist_r[:, e0 - 0:e0 + P][:, -P:])
            zT = sb.tile([NB, P], F32, tag="zT", name=uid("zT"))
            nc.vector.scalar_tensor_tensor(zT[:], d_rep[:], rbf_A[:, :1],
                                           ap3(rbf_B, [[0, P]]),
                                           op0=OP.mult, op1=OP.add)
            zsqT = sb.tile([NB, P], F32, tag="zsqT", name=uid("zq"))
            nc.scalar.square(zsqT[:], zT[:])
            esT = sb.tile([NB, P], BF16, tag="esT", name=uid("es"))
            nc.scalar.activation(esT[:], zsqT[:], ACTF.Exp, scale=-0.5)

            # radial first matmul; evac PSUM to SBUF so the 8 LN chains can
            # run stage-major without holding PSUM
            x1 = ps.tile([P, 128], F32, tag="ps_small", name=uid("x1"))
            nc.tensor.matmul(x1[:], esT[:], W1p[:], start=True, stop=True)
            x1s_ = sbx.tile([P, 128], F32, tag="x1sb", name=uid("x1s"))
            nc.scalar.copy(x1s_[:], x1[:])
            x1sb.append(x1s_)

        # ---- radial stage-major: 8 chains = (subtile, nf|ef) ----
        chains = [(s, ri, p) for s in range(NSUB)
                  for ri, p in enumerate(('nf', 'ef'))]
        h1s = ln_silu_stage(
            [x1sb[s][:, 64 * ri:64 * ri + 64] for (s, ri, p) in chains],
            [(p + '_g1', p + '_b1') for (s, ri, p) in chains], 64, sbc)
        x2sb = []
        for i, (s, ri, p) in enumerate(chains):
            h1T_p = psb.tile([64, P], BF16, tag="ps_bf", name=uid("h1t"))
            nc.tensor.transpose(h1T_p[:], h1s[i][:], identB[:])
            h1T = sbc.tile([64, P], BF16, tag="h1T", name=uid("h1T"))
            nc.scalar.copy(h1T[:], h1T_p[:])
            x2 = ps.tile([P, 64], F32, tag="ps_small", name=uid("x2"))
            nc.tensor.matmul(x2[:], h1T[:], (W2nf if p == 'nf' else W2ef)[:],
                             start=True, stop=True)
            x2s_ = sbc.tile([P, 64], F32, tag="x2sb", name=uid("x2s"))
            nc.scalar.copy(x2s_[:], x2[:])
            x2sb.append(x2s_)
        h2s = ln_silu_stage(
            [x2sb[i][:, :] for i in range(8)],
            [(p + '_g2', p + '_b2') for (s, ri, p) in chains], 64, sbc)
        for i, (s, ri, p) in enumerate(chains):
            h2T_p = psb.tile([64, P], BF16, tag="ps_bf", name=uid("h2t"))
            nc.tensor.transpose(h2T_p[:], h2s[i][:], identB[:])
            nc.scalar.copy(h2T[p][:, s * P:s * P + P], h2T_p[:])

        # ---- edge transform (transposed directly) ----
        pes = psm.tile([64, DT], F32, tag="ps_med", name=uid("pes"))
        nc.tensor.matmul(pes[:], etWs[:], heT[:], start=True, stop=True)
        esE = sbT.tile([64, DT], BF16, tag="esE", name=f"esE_{td}")
        nc.vector.tensor_scalar(esE[:], pes[:], bcols[0:64, 3:4], None, op0=OP.add)
        pev = psm.tile([96, DT], F32, tag="ps_med", name=uid("pev"))
        for x in range(3):
            nc.tensor.matmul(pev[32 * x:32 * x + 32, :], etWv[:], hevT[x][:],
                             start=True, stop=True, skip_group_check=True)
        evE = sbT.tile([96, DT], BF16, tag="evE", name=f"evE_{td}")
        nc.scalar.copy(evE[:], pev[:])

        # ---- replicas for the transposed muls ----
        def rep64(src_t, nm):
            r = sbT.tile([128, DT], BF16, tag=nm, name=f"{nm}_{td}")
            nc.vector.tensor_copy(r[0:64, :], src_t[:, :])
            nc.vector.tensor_copy(r[64:128, :], src_t[:, :])
            return r

        def rep32(src_t, x, nm):
            rp = psw.tile([128, DT], F32, tag="psw", name=uid("rp"))
            nc.tensor.matmul(rp[:], R96[:, 128 * x:128 * (x + 1)], src_t[:],
                             start=True, stop=True)
            r = sbT.tile([128, DT], BF16, tag=nm, name=f"{nm}_{td}")
            nc.scalar.copy(r[:], rp[:])
            return r

        s2rep = rep64(s2T, "s2rep")
        esrep = rep64(esE, "esrep")
        v2rep = [rep32(v2T, x, f"v2rep{x}") for x in range(3)]
        evrep = [rep32(evE, x, f"evrep{x}") for x in range(3)]

        # ---- dtp path driver (transposed) ----
        _evac_alt = [0]

        def path(W3t, colbase, nblocks, zreps, UB, uw, outsec):
            """UB: packed one-hot selectors, block b = cols [uw*b, uw*(b+1));
            outsec(xi) -> (psum_tile, base_row). Accumulates over blocks.
            Software-pipelined by one block so PE's reduce-matmul of block b
            doesn't sit in front of block b+1's weight-matmul in program
            order. w-evacs alternate Act/Pool to balance engine load."""
            pend = []
            for b in range(nblocks):
                pw = psw.tile([128, DT], F32, tag="psw", name=uid("pw"))
                nc.tensor.matmul(pw[:], W3t[:, colbase + 128 * b:colbase + 128 * (b + 1)],
                                 (h2T['nf'] if W3t is W3nf else h2T['ef'])[:],
                                 start=True, stop=True)
                single = len(zreps) == 1
                pool_blk = single and b % 4 == 3
                if not single or pool_blk:
                    wsb = sbq.tile([128, DT], BF16, tag="wsb", name=uid("w"))
                    nc.scalar.copy(wsb[:], pw[:])
                qs = []
                for xi, zr in enumerate(zreps):
                    q = sbQ.tile([128, DT], BF16, tag="q", name=uid("q"))
                    if single and not pool_blk:
                        nc.vector.tensor_tensor(q[:], pw[:], zr[:], op=OP.mult)
                    elif pool_blk or xi == 2:
                        nc.gpsimd.tensor_tensor(q[:], wsb[:], zr[:], op=OP.mult)
                    else:
                        nc.vector.tensor_tensor(q[:], wsb[:], zr[:], op=OP.mult)
                    qs.append((xi, q))
                for xi, q in pend:
                    pt, row0 = outsec(xi)
                    nc.tensor.matmul(pt[row0:row0 + uw, :],
                                     UB[:, uw * (b - 1):uw * b], q[:],
                                     start=(b - 1 == 0), stop=False,
                                     skip_group_check=True)
                pend = qs
            for xi, q in pend:
                pt, row0 = outsec(xi)
                nc.tensor.matmul(pt[row0:row0 + uw, :],
                                 UB[:, uw * (nblocks - 1):uw * nblocks], q[:],
                                 start=(nblocks == 1), stop=True,
                                 skip_group_check=True)

        def evac(pt, w, nm):
            t = sbT.tile([w, DT], BF16, tag=nm, name=f"{nm}_{td}")
            nc.scalar.copy(t[:], pt[0:w, :])
            return t

        # dtp1 paths
        p_ss = psp.tile([64, DT], F32, tag="psp", name=uid("pss"))
        path(W3nf, 0, 32, [s2rep], UBss, 64, lambda xi: (p_ss, 0))
        bil1 = evac(p_ss, 64, "bil1")
        p_vs = psp.tile([64, DT], F32, tag="psp", name=uid("pvs"))
        path(W3nf, 6144, 16, [s2rep], UBss, 64, lambda xi: (p_vs, 0))
        bvsR = sbT.tile([96, DT], BF16, tag="bvsR", name=f"bvsR_{td}")
        for x in range(3):
            nc.scalar.copy(bvsR[32 * x:32 * x + 32, :], p_vs[0:32, :])
        p_svA = psp.tile([128, DT], F32, tag="psp", name=uid("psa"))
        p_svB = psp.tile([64, DT], F32, tag="psp", name=uid("psb2"))
        path(W3nf, 4096, 16, v2rep, UBsv, 64,
             lambda xi: (p_svA, 64 * xi) if xi < 2 else (p_svB, 0))
        bsv = []
        for x in range(3):
            t = sbT.tile([64, DT], BF16, tag=f"bsv{x}", name=f"bsv{x}_{td}")
            nc.scalar.copy(t[:], p_svA[64 * x:64 * x + 64, :] if x < 2
                           else p_svB[0:64, :])
            bsv.append(t)
        p_v0 = psp.tile([96, DT], F32, tag="psp", name=uid("pv0"))
        path(W3nf, 8192, 8, v2rep, UB4, 32, lambda xi: (p_v0, 32 * xi))
        bv0 = evac(p_v0, 96, "bv0")
        p_v1 = psp.tile([96, DT], F32, tag="psp", name=uid("pv1"))
        path(W3nf, 9216, 8, v2rep, UB4, 32, lambda xi: (p_v1, 32 * xi))
        # cbuf in two section rotations: P1[s]=cbuf[(s+1)%3], P2[s]=cbuf[(s+2)%3]
        cbufP1 = sbT.tile([96, DT], BF16, tag="cbufP1", name=f"cbufP1_{td}")
        cbufP2 = sbT.tile([96, DT], BF16, tag="cbufP2", name=f"cbufP2_{td}")
        for s_ in range(3):
            nc.scalar.copy(cbufP1[32 * s_:32 * s_ + 32, :],
                           p_v1[32 * ((s_ + 1) % 3):32 * ((s_ + 1) % 3) + 32, :])
            nc.scalar.copy(cbufP2[32 * s_:32 * s_ + 32, :],
                           p_v1[32 * ((s_ + 2) % 3):32 * ((s_ + 2) % 3) + 32, :])

        # ---- epilogue 1: fsT [96, DT], fvT_x [128, DT] (bf16) ----
        fsT = sbT.tile([96, DT], BF16, tag="fsT", name=f"fsT_{td}")
        nc.vector.tensor_tensor(fsT[0:64, :], bil1[:], s1T[:], op=OP.mult)
        t96 = sbT.tile([96, DT], BF16, tag="t96", name=f"t96_{td}")
        nc.vector.tensor_tensor(t96[:], v1T[:], bv0[:], op=OP.mult)
        p_x = psm.tile([32, DT], F32, tag="ps_med", name=uid("px"))
        nc.tensor.matmul(p_x[:], U3[:], t96[:], start=True, stop=True)
        nc.scalar.copy(fsT[64:96, :], p_x[:])
        nc.vector.tensor_scalar(fsT[:], fsT[:], bcols[0:96, 0:1], None, op0=OP.add)

        # cross products, all base-aligned: ta[s] = v1T[s]*cbuf[(s+1)%3],
        # tb[s] = v1T[s]*cbuf[(s+2)%3]; cross_x = ta[(x+1)%3] - tb[(x+2)%3]
        ta_all = sbq.tile([96, DT], BF16, tag="ta", name=uid("ta"))
        nc.vector.tensor_tensor(ta_all[:], v1T[:], cbufP1[:], op=OP.mult)
        tb_all = sbq.tile([96, DT], BF16, tag="tb", name=uid("tb"))
        nc.vector.tensor_tensor(tb_all[:], v1T[:], cbufP2[:], op=OP.mult)
        # rotate tb by +1 section so cross_all[s] = ta_all[s] - tb_rot[s]
        # with cross_all[(x+1)%3] = cross_x
        tb_rot = sbq.tile([96, DT], BF16, tag="tbr", name=uid("tbr"))
        for s_ in range(3):
            nc.vector.tensor_copy(tb_rot[32 * s_:32 * s_ + 32, :],
                                  tb_all[32 * ((s_ + 1) % 3):32 * ((s_ + 1) % 3) + 32, :])
        cross_all = sbq.tile([96, DT], BF16, tag="cra", name=uid("cr"))
        nc.vector.tensor_sub(cross_all[:], ta_all[:], tb_rot[:])
        fvT = []
        for x in range(3):
            fv = sbT.tile([128, DT], BF16, tag=f"fvT{x}", name=f"fvT{x}_{td}")
            nc.vector.tensor_tensor(fv[0:64, :], bsv[x][:], s1T[:], op=OP.mult)
            nc.vector.tensor_tensor(fv[64:96, :], v1T[32 * x:32 * x + 32, :],
                                    bvsR[32 * x:32 * x + 32, :], op=OP.mult)
            yx = (x + 1) % 3
            nc.vector.tensor_copy(fv[96:128, :], cross_all[32 * yx:32 * yx + 32, :])
            fvT.append(fv)

        # ---- node-fusion linear (reads transposed directly) ----
        pns = psm.tile([64, DT], F32, tag="ps_med", name=uid("pns"))
        nc.tensor.matmul(pns[:], ntWs[:], fsT[:], start=True, stop=True)
        nsT = sbT.tile([64, DT], BF16, tag="nsT", name=f"nsT_{td}")
        nc.vector.tensor_scalar(nsT[:], pns[:], bcols[0:64, 2:3], None, op0=OP.add)
        pnv = psm.tile([96, DT], F32, tag="ps_med", name=uid("pnv"))
        for x in range(3):
            nc.tensor.matmul(pnv[32 * x:32 * x + 32, :], ntWv[:], fvT[x][:],
                             start=True, stop=True, skip_group_check=True)
        nvT = sbT.tile([96, DT], BF16, tag="nvT", name=f"nvT_{td}")
        nc.scalar.copy(nvT[:], pnv[:])

        # ---- dtp2 ----
        p_ss2 = psp.tile([64, DT], F32, tag="psp", name=uid("ps2"))
        path(W3ef, 0, 32, [esrep], UBss, 64, lambda xi: (p_ss2, 0))
        bil2 = evac(p_ss2, 64, "bil2")
        p_v02 = psp.tile([96, DT], F32, tag="psp", name=uid("pv2"))
        path(W3ef, 4096, 8, evrep, UB4, 32, lambda xi: (p_v02, 32 * xi))
        bv02 = evac(p_v02, 96, "bv02")

        asT = sbT.tile([96, DT], BF16, tag="asT", name=f"asT_{td}")
        nc.vector.tensor_tensor(asT[0:64, :], bil2[:], nsT[:], op=OP.mult)
        t96b = sbT.tile([96, DT], BF16, tag="t96b", name=f"t96b_{td}")
        nc.vector.tensor_tensor(t96b[:], nvT[:], bv02[:], op=OP.mult)
        p_x2 = psm.tile([32, DT], F32, tag="ps_med", name=uid("px2"))
        nc.tensor.matmul(p_x2[:], U3[:], t96b[:], start=True, stop=True)
        nc.scalar.copy(asT[64:96, :], p_x2[:])
        nc.vector.tensor_scalar(asT[:], asT[:], bcols[0:96, 1:2], None, op0=OP.add)

        # defer adaLN/head/scatter: collect per-512 state
        asTs.append(asT)
        g_modA += g_mod; d_colsA += d_cols; rvsA += rvs; srcfsA += srcfs

    # ---- deferred tail, stage-major across all 8 subtiles ----
    NS8 = NDT * NSUB
    as_l = []
    for k in range(NS8):
        td, s = divmod(k, NSUB)
        as_p = psb.tile([P, S_TP], BF16, tag="ps_bf", name=uid("asp"))
        nc.tensor.transpose(as_p[:], asTs[td][:, s * P:s * P + P],
                            identB[0:96, 0:96])
        as_ = sbc.tile([P, S_TP], F32, tag="as", name=uid("as"))
        nc.scalar.copy(as_[:], as_p[:])
        as_l.append(as_)
    mus = [sbc.tile([P, 1], F32, tag="amu", name=uid("am")) for _ in range(NS8)]
    for k in range(NS8):
        nc.vector.tensor_reduce(mus[k][:], as_l[k][:], axis=AX.X, op=OP.add)
    for k in range(NS8):
        nc.vector.tensor_scalar_mul(mus[k][:], mus[k][:], 1.0 / S_TP)
    cens = [sbc.tile([P, S_TP], F32, tag="acen", name=uid("ac"))
            for _ in range(NS8)]
    for k in range(NS8):
        nc.vector.tensor_scalar(cens[k][:], as_l[k][:], mus[k][:, :1], None,
                                op0=OP.subtract)
    vars_ = [sbc.tile([P, 1], F32, tag="avar", name=uid("av"))
             for _ in range(NS8)]
    for k in range(NS8):
        sqv = sb.tile([P, S_TP], F32, tag="asq", name=uid("aq"))
        nc.scalar.activation(sqv[:], cens[k][:], ACTF.Square,
                             accum_out=vars_[k][:])
    rstds = []
    for k in range(NS8):
        rstds.append(rstd_from_var(vars_[k][:], 1.0 / S_TP, sbc))
    s_ns = [sbc.tile([P, S_TP], BF16, tag="sn", name=uid("sn"))
            for _ in range(NS8)]
    for k in range(NS8):
        nc.vector.scalar_tensor_tensor(s_ns[k][:], cens[k][:],
                                       rstds[k][:, :1],
                                       g_modA[k][:, S_TP:2 * S_TP],
                                       op0=OP.mult, op1=OP.mult)
    for k in range(NS8):
        nc.vector.tensor_tensor(s_ns[k][:], s_ns[k][:], g_modA[k][:, 0:S_TP],
                                op=OP.add)
    hds = []
    for k in range(NS8):
        snT_p = psb.tile([S_TP, P], BF16, tag="ps_bf", name=uid("snp"))
        nc.tensor.transpose(snT_p[:], s_ns[k][:], identB[:])
        snT = sbc.tile([S_TP, P], BF16, tag="snT", name=uid("snT"))
        nc.scalar.copy(snT[:], snT_p[:])
        hd_p = ps.tile([P, 32], F32, tag="ps_small", name=uid("hdp"))
        nc.tensor.matmul(hd_p[:], snT[:], spW1[:], start=True, stop=True)
        hd = sbc.tile([P, 32], F32, tag="hd", name=uid("hd"))
        nc.vector.tensor_tensor(hd[:], hd_p[:], rows_slice(rep, 'sp_b1'),
                                op=OP.add)
        hds.append(hd)
    es_ = [sbc.tile([P, 32], F32, tag="hse", name=uid("he")) for _ in range(NS8)]
    for k in range(NS8):
        nc.scalar.activation(es_[k][:], hds[k][:], ACTF.Exp, scale=-1.0)
    for k in range(NS8):
        nc.vector.tensor_scalar_add(es_[k][:], es_[k][:], 1.0)
    rs_ = [sbc.tile([P, 32], F32, tag="hsr", name=uid("hr")) for _ in range(NS8)]
    for k in range(NS8):
        nc.vector.reciprocal(rs_[k][:], es_[k][:])
    siles = [sbc.tile([P, 32], F32, tag="sile", name=uid("sl"))
             for _ in range(NS8)]
    for k in range(NS8):
        nc.vector.tensor_tensor(siles[k][:], hds[k][:], rs_[k][:], op=OP.mult)
    sws = [sbc.tile([P, 1], F32, tag="swv", name=uid("sv")) for _ in range(NS8)]
    for k in range(NS8):
        swt = sb.tile([P, 32], F32, tag="swt", name=uid("sw"))
        nc.vector.tensor_tensor(swt[:], siles[k][:], rows_slice(rep, 'spW2r'),
                                op=OP.mult)
        nc.vector.tensor_reduce(sws[k][:], swt[:], axis=AX.X, op=OP.add)
        nc.vector.tensor_scalar(sws[k][:], sws[k][:], 32.0 ** -0.5,
                                rep[:, ROWS['sp_b2'][0]:ROWS['sp_b2'][0] + 1],
                                op0=OP.mult, op1=OP.add)
    forces = []
    for k in range(NS8):
        den = sbc.tile([P, 1], F32, tag="den", name=uid("dn"))
        nc.vector.scalar_tensor_tensor(den[:], d_colsA[k], 1.0, d_colsA[k],
                                       op0=OP.add, op1=OP.mult)
        rden = sbc.tile([P, 1], F32, tag="rden", name=uid("rd"))
        nc.vector.reciprocal(rden[:], den[:])
        coef = sbc.tile([P, 1], F32, tag="coef", name=uid("cf"))
        nc.vector.tensor_mul(coef[:], sws[k][:], rden[:])
        force = sbc.tile([P, 3], BF16, tag="force", name=uid("fo"))
        nc.vector.tensor_scalar(force[:], rvsA[k], coef[:, :1], None,
                                op0=OP.mult)
        forces.append(force)

    # scatter: one PSUM accumulator over all 8 subtiles
    acc_p = psp.tile([P, NCHUNK * 3], F32, tag="psp", name=uid("ap"))
    for ch in range(NCHUNK):
        ohs = []
        for k in range(NS8):
            ssh = sbc.tile([P, 1], F32, tag="ssh", name=uid("sh"))
            nc.vector.tensor_scalar_add(ssh[:], srcfsA[k], float(-P * ch))
            oh = sbc.tile([P, P], BF16, tag="oh", name=uid("oh"))
            nc.gpsimd.tensor_scalar(oh[:], iota_f[:], ssh[:, :1], None,
                                    op0=OP.is_equal)
            ohs.append(oh)
        for k in range(NS8):
            nc.tensor.matmul(acc_p[:, 3 * ch:3 * ch + 3], ohs[k][:],
                             forces[k][:],
                             start=(k == 0), stop=(k == NS8 - 1),
                             skip_group_check=True)
    acc_sb = consts.tile([P, NCHUNK * 3], F32)
    nc.scalar.copy(acc_sb[:], acc_p[:])
    dma(T['out'][:], acc_sb[:])


# ======================= host side =======================

def _bf16(a):
    return np.asarray(a, np.float32).astype(mybir.dt.np(BF16))


def host_prep(inp):
    inp = {k: np.asarray(v) for k, v in inp.items()}
    src = inp['edge_index'][0].astype(np.int32)
    dst = inp['edge_index'][1].astype(np.int32)
    perm = np.argsort(src, kind='stable')
    src, dst = src[perm], dst[perm]
    gid = inp['batch'].astype(np.int32)[src]
    h_edge = inp['h_edge'][perm]
    dist = inp['distance'][perm].astype(np.float32)
    rvec = inp['relative_vec'][perm].astype(np.float32)

    rows = np.zeros(RWID, np.float32)

    def setr(name, val):
        off, w = ROWS[name]
        rows[off:off + w] = val
    for p in ('nf', 'ef'):
        for q in ('g1', 'b1', 'g2', 'b2'):
            setr(f'{p}_{q}', inp[f'{p}_{q}'])
    setr('src_bs', inp['src_bs']); setr('dst_bs', inp['dst_bs'])
    setr('sp_b1', inp['sp_b1']); setr('spW2r', inp['sp_W2'][:, 0])
    rows[ROWS['sp_b2'][0]] = inp['sp_b2'][0]
    rows[ROWS['eps'][0]] = 1e-5
    setr('normbt', inp['norm_bt'][:2 * S_TP])

    std = np.asarray(inp['rbf_std'], np.float32)
    mean = np.asarray(inp['rbf_mean'], np.float32)
    rbf_w = float(np.asarray(inp['rbf_w'])); rbf_b = float(np.asarray(inp['rbf_b']))
    A = (rbf_w / (CUTOFF * std)).reshape(-1, 1).astype(np.float32)
    Bc = ((rbf_b - mean) / std).reshape(-1, 1).astype(np.float32)
    cnorm = (1.0 / (np.sqrt(2 * np.pi) * std)).reshape(-1, 1).astype(np.float32)
    W1p = np.concatenate([inp['nf_W1'], inp['ef_W1']], axis=1) * cnorm

    def fold_w3(W3):
        W3 = np.asarray(W3, np.float32).copy()
        W3[:, 0:4096] *= 64.0 ** -0.5
        W3[:, 4096:6144] *= 32.0 ** -0.5
        W3[:, 6144:8192] *= 64.0 ** -0.5
        W3[:, 8192:9216] *= 96.0 ** -0.5
        W3[:, 9216:10240] *= 64.0 ** -0.5
        return W3
    W3nf = fold_w3(inp['nf_W3'])
    W3ef_f = fold_w3(inp['ef_W3'])
    W3ef = np.concatenate([W3ef_f[:, :4096], W3ef_f[:, 8192:9216]], axis=1)

    Wsd = np.concatenate([inp['src_Ws'], inp['dst_Ws']], axis=1) * 128.0 ** -0.5
    Wvsd = np.concatenate([inp['src_Wv'], inp['dst_Wv']], axis=1) * 64.0 ** -0.5

    pp = np.arange(128)
    # UBss: block b (of 32) -> row 2b + p//64, packed as cols [64b, 64b+64)
    UBss = np.zeros((128, 2048), np.float32)
    for b in range(32):
        UBss[pp, 64 * b + np.minimum(2 * b + pp // 64, 63)] = (2 * b + pp // 64 < 64)
    # UBsv: block b (of 16) -> row 4b + p//32, cols [64b, 64b+64)
    UBsv = np.zeros((128, 1024), np.float32)
    for b in range(16):
        UBsv[pp, 64 * b + 4 * b + pp // 32] = 1.0
    # UB4: block b (of 8) -> row 4b + p//32, cols [32b, 32b+32)
    UB4 = np.zeros((128, 256), np.float32)
    for b in range(8):
        UB4[pp, 32 * b + 4 * b + pp // 32] = 1.0
    U3 = (np.arange(96)[:, None] % 32 == np.arange(32)[None, :]).astype(np.float32)
    R96 = np.zeros((96, 384), np.float32)
    for x in range(3):
        R96[:, 128 * x:128 * (x + 1)] = (
            np.arange(96)[:, None] == 32 * x + np.arange(128)[None, :] % 32)
    bcols = np.zeros((96, 4), np.float32)
    bcols[:, 0] = inp['nf_bias']; bcols[:, 1] = inp['ef_bias']
    bcols[:64, 2] = inp['nt_bs']; bcols[:64, 3] = inp['et_bs']

    # device row order: scalars then x-grouped vector components
    hnT = np.ascontiguousarray(inp['h_node'].T)          # [320, N]
    hnT = np.concatenate([hnT[0:128], hnT[128::3], hnT[129::3], hnT[130::3]], 0)
    shared = dict(
        hn_T=_bf16(hnT),
        t_T=_bf16(np.ascontiguousarray(inp['t'].T)),
        normWt=_bf16(np.ascontiguousarray(inp['norm_Wt'][:, :2 * S_TP])),
        rows=np.ascontiguousarray(rows.reshape(1, -1), np.float32),
        rbf_A=A, rbf_B=Bc,
        W1p=_bf16(W1p),
        nf_W2=_bf16(inp['nf_W2']), ef_W2=_bf16(inp['ef_W2']),
        W3nf=_bf16(W3nf), W3ef=_bf16(W3ef),
        Wsd=_bf16(Wsd), Wvsd=_bf16(Wvsd),
        nt_Ws=_bf16(inp['nt_Ws'] * 96.0 ** -0.5),
        nt_Wv=_bf16(inp['nt_Wv'] * 128.0 ** -0.5),
        et_Ws=_bf16(inp['et_Ws'] * 64.0 ** -0.5),
        et_Wv=_bf16(inp['et_Wv'] * 32.0 ** -0.5),
        sp_W1=_bf16(inp['sp_W1'] * 96.0 ** -0.5),
        UBss=_bf16(UBss), UBsv=_bf16(UBsv), UB4=_bf16(UB4),
        U3=_bf16(U3), R96=_bf16(R96),
        bcols=bcols,
    )

    in_maps = []
    for c in range(NC_CORES):
        sl = slice(c * EC, (c + 1) * EC)
        m = dict(shared)
        heT_ = np.ascontiguousarray(h_edge[sl].T)        # [160, EC]
        heT_ = np.concatenate([heT_[0:64], heT_[64::3], heT_[65::3], heT_[66::3]], 0)
        m['he_T'] = _bf16(heT_)
        m['dist_r'] = np.ascontiguousarray(dist[sl].reshape(1, -1))
        m['edgef'] = np.ascontiguousarray(np.concatenate(
            [dist[sl, None], rvec[sl], src[sl, None].astype(np.float32)],
            axis=1), np.float32)
        m['edgei'] = np.ascontiguousarray(np.stack(
            [src[sl], dst[sl], gid[sl]], axis=1).astype(np.int32))
        in_maps.append(m)
    return in_maps


_CACHED_NC = None


def kernel(**inputs):
    global _CACHED_NC
    from concourse.bass_utils import run_bass_kernel_spmd
    if _CACHED_NC is None:
        _CACHED_NC = build_nc()
    in_maps = host_prep(inputs)
    res = run_bass_kernel_spmd(_CACHED_NC, in_maps, list(range(NC_CORES)))
    out = np.zeros((128, NCHUNK, 3), np.float32)
    for r in res.results:
        out += r['out'].reshape(128, NCHUNK, 3)
    return np.ascontiguousarray(out.transpose(1, 0, 2).reshape(N, 3))


# revision 58
# speedup vs baseline: 1.0909x; 1.0909x over previous
"""Bass/Trainium2 kernel for nn_EquivariantPosUpdate — 8-core edge-parallel.

v2: transposed depthwise-TP. Per core, 1024 edges in 2 dtp-tiles of DT=512
(4 subtiles of 128 for gather/RBF/radial/adaLN work that needs edges on
partitions).

Key ideas vs v1:
  - bf16 everywhere PE touches (4x matmul, 2x DVE) with scales folded on host
  - TP weights computed TRANSPOSED: w^T[(u,v), e] = matmul(W3-block, h2T),
    so the v-contraction becomes a one-hot PE matmul (PSUM row writes)
    instead of a DVE TensorReduce — the v1 bottleneck (238us of DVE reduces)
    moves to the 4x-faster, underutilized PE
  - all epilogues run in [feat, e] layout; nt/et linears consume them
    directly as lhsT (no fs/fv transposes)
  - one activation table (exp/ln/square/copy): rstd = exp(-0.5*ln(var)),
    silu via exp + fast-reciprocal on DVE
"""
import sys
sys.path.insert(0, '/opt/trn_rl_repo')
import numpy as np
from contextlib import ExitStack

import concourse.bass as bass
import concourse.bacc as bacc
import concourse.mybir as mybir
import concourse.tile as tile
from concourse.bass import AP, IndirectOffsetOnAxis
from concourse.masks import make_identity

F32 = mybir.dt.float32
BF16 = mybir.dt.bfloat16
I32 = mybir.dt.int32
AX = mybir.AxisListType
OP = mybir.AluOpType
ACTF = mybir.ActivationFunctionType

N, E, G, NB = 2048, 8192, 64, 128
NC_CORES = 8
EC = E // NC_CORES          # 1024
P = 128
DT = 512                    # dtp tile (edges on free axis)
NDT = EC // DT              # 2
NSUB = DT // P              # 4
M0, M1 = 64, 32
S_TP = 96
CUTOFF = 5.0
NCHUNK = N // P             # 16

ROWS = {}
_off = 0
for _n, _w in [('nf_g1', 64), ('nf_b1', 64), ('nf_g2', 64), ('nf_b2', 64),
               ('ef_g1', 64), ('ef_b1', 64), ('ef_g2', 64), ('ef_b2', 64),
               ('src_bs', 64), ('dst_bs', 64),
               ('sp_b1', 32), ('spW2r', 32),
               ('sp_b2', 1), ('eps', 1), ('normbt', 192)]:
    ROWS[_n] = (_off, _w)
    _off += _w
RWID = _off


def rows_slice(rep, name):
    off, w = ROWS[name]
    return rep[:, off:off + w]


def ap3(t, dims, offset=0):
    base = t[:, :] if not isinstance(t, AP) else t
    ap = AP(base.tensor, base.offset + offset, [base.ap[0]] + [list(d) for d in dims])
    return ap


def build_nc():
    nc = bacc.Bacc("TRN2", target_bir_lowering=False, debug=False,
                   num_devices=NC_CORES)
    T = {}

    def din(name, shape, dtype=F32):
        T[name] = nc.dram_tensor(name, shape, dtype, kind="ExternalInput")
        return T[name]

    din('hn_T', [320, N], BF16); din('he_T', [160, EC], BF16)
    din('dist_r', [1, EC])
    din('edgef', [EC, 5]); din('edgei', [EC, 3], I32)
    din('t_T', [128, G], BF16); din('normWt', [128, 2 * S_TP], BF16)
    din('rows', [1, RWID])
    din('rbf_A', [NB, 1]); din('rbf_B', [NB, 1])
    din('W1p', [NB, 128], BF16)
    din('nf_W2', [64, 64], BF16); din('ef_W2', [64, 64], BF16)
    din('W3nf', [64, 10240], BF16); din('W3ef', [64, 5120], BF16)
    din('Wsd', [128, 128], BF16); din('Wvsd', [64, 64], BF16)
    din('nt_Ws', [S_TP, 64], BF16); din('nt_Wv', [128, 32], BF16)
    din('et_Ws', [64, 64], BF16); din('et_Wv', [32, 32], BF16)
    din('sp_W1', [S_TP, 32], BF16)
    din('UBss', [128, 2048], BF16); din('UBsv', [128, 1024], BF16)
    din('UB4', [128, 256], BF16)
    din('U3', [96, 32], BF16); din('R96', [96, 384], BF16)
    din('bcols', [96, 4])
    out = nc.dram_tensor('out', [128, NCHUNK * 3], F32, kind="ExternalOutput")
    T['out'] = out
    T['proj_src'] = nc.dram_tensor('proj_src', [N, 160], BF16)
    T['proj_dst'] = nc.dram_tensor('proj_dst', [N, 160], BF16)
    T['mod_d'] = nc.dram_tensor('mod_d', [G, 2 * S_TP], F32)

    with tile.TileContext(nc) as tc:
        with ExitStack() as ctx:
            _build(ctx, tc, nc, T)
    nc.compile()
    return nc


def _build(ctx, tc, nc, T):
    consts = ctx.enter_context(tc.tile_pool(name="consts", bufs=1))
    setup = ctx.enter_context(tc.tile_pool(name="setup", bufs=2))
    sb = ctx.enter_context(tc.tile_pool(name="sb", bufs=3))
    sbq = ctx.enter_context(tc.tile_pool(name="sbq", bufs=3))
    sbQ = ctx.enter_context(tc.tile_pool(name="sbQ", bufs=6))
    sbg = ctx.enter_context(tc.tile_pool(name="sbg", bufs=2))
    sbT = ctx.enter_context(tc.tile_pool(name="sbT", bufs=2))
    sbc = ctx.enter_context(tc.tile_pool(name="sbc", bufs=8))
    sbx = ctx.enter_context(tc.tile_pool(name="sbx", bufs=4))
    ps = ctx.enter_context(tc.tile_pool(name="ps", bufs=2, space="PSUM"))
    psm = ctx.enter_context(tc.tile_pool(name="psm", bufs=1, space="PSUM"))
    psb = ctx.enter_context(tc.tile_pool(name="psb", bufs=1, space="PSUM"))
    psw = ctx.enter_context(tc.tile_pool(name="psw", bufs=2, space="PSUM"))
    psp = ctx.enter_context(tc.tile_pool(name="psp", bufs=2, space="PSUM"))
    dma = nc.sync.dma_start

    def load(name, pool=consts):
        t = pool.tile(T[name].shape, T[name].dtype, tag="ld_" + name,
                      name="ld_" + name)
        dma(t[:], T[name][:])
        return t

    # ---------------- setup ----------------
    # pin one act table (exp/ln/square/copy) so the table-load pass never
    # needs to swap between exp-only and ln-only sets
    PIN_ACT_TABLE = True
    if PIN_ACT_TABLE:
        from concourse.hw_specs import get_activation_tables
        _tabs = list(get_activation_tables(nc.m.arch))
        _tid = _tabs.index('natural_log_exp_and_others')
        nc.scalar.add_instruction(mybir.InstLoadActFuncSet(
            name=nc.get_next_instruction_name(), ins=[], outs=[],
            act_func_set_id=_tid))
    ident = consts.tile([P, P], F32)
    make_identity(nc, ident[:])
    identB = consts.tile([P, P], BF16)
    nc.vector.tensor_copy(identB[:], ident[:])
    iota_i = consts.tile([P, P], I32)
    nc.gpsimd.iota(iota_i[:], pattern=[[1, P]], base=0, channel_multiplier=0)
    iota_f = consts.tile([P, P], F32)
    nc.vector.tensor_copy(iota_f[:], iota_i[:])

    rows1 = consts.tile([1, RWID], F32)
    dma(rows1[:], T['rows'][:])
    o_nbt = ROWS['normbt'][0]
    nc.vector.tensor_scalar_add(rows1[:, o_nbt + S_TP:o_nbt + 2 * S_TP],
                                rows1[:, o_nbt + S_TP:o_nbt + 2 * S_TP], 1.0)
    rep = consts.tile([P, RWID], F32)
    nc.gpsimd.partition_broadcast(rep[:], rows1[:])

    Wsd = load('Wsd'); Wvsd = load('Wvsd')
    WvsdHi = consts.tile([128, 64], BF16, tag="wvhi")
    nc.vector.tensor_copy(WvsdHi[64:128, :], Wvsd[:, :])
    rbf_A = load('rbf_A'); rbf_B = load('rbf_B')
    dist_r = load('dist_r')
    W1p = load('W1p')
    W2nf = load('nf_W2'); W2ef = load('ef_W2')
    ntWs = load('nt_Ws'); ntWv = load('nt_Wv')
    etWs = load('et_Ws'); etWv = load('et_Wv')
    spW1 = load('sp_W1')
    normWt = load('normWt'); tT = load('t_T')
    U3 = load('U3'); R96 = load('R96')
    bcols = load('bcols')
    UBss = load('UBss'); UBsv = load('UBsv'); UB4 = load('UB4')
    W3nf = consts.tile(T['W3nf'].shape, BF16, tag="ld_W3nf", name="ld_W3nf")
    nc.scalar.dma_start(W3nf[:], T['W3nf'][:])
    W3ef = consts.tile(T['W3ef'].shape, BF16, tag="ld_W3ef", name="ld_W3ef")
    nc.scalar.dma_start(W3ef[:], T['W3ef'][:])

    _uid = [0]

    def uid(tag):
        _uid[0] += 1
        return f"{tag}_{_uid[0]}"

    def silu(dst, src_ap, width, pool):
        e_ = pool.tile([P, width], F32, tag="silu_e", name=uid("se"))
        nc.scalar.activation(e_[:], src_ap, ACTF.Exp, scale=-1.0)
        d_ = pool.tile([P, width], F32, tag="silu_d", name=uid("sd"))
        nc.vector.tensor_scalar_add(d_[:], e_[:], 1.0)
        r_ = pool.tile([P, width], F32, tag="silu_r", name=uid("sr"))
        nc.vector.reciprocal_approx_fast(out=r_[:], in_=d_[:])
        nc.vector.tensor_tensor(dst, src_ap, r_[:], op=OP.mult)

    def rstd_from_var(var_ap, inv_width, pool):
        lnv = pool.tile([P, 1], F32, tag="lnv", name=uid("lnv"))
        nc.scalar.activation(lnv[:], var_ap, ACTF.Ln, scale=inv_width,
                             bias=rep[:, ROWS['eps'][0]:ROWS['eps'][0] + 1])
        rstd = pool.tile([P, 1], F32, tag="rstd", name=uid("rstd"))
        nc.scalar.activation(rstd[:], lnv[:], ACTF.Exp, scale=-0.5)
        return rstd

    # ---------------- phase A: node projections (bf16) ----------------
    # hn_T host row order: [s(128) | x0(64) | x1(64) | x2(64)] so each load
    # is contiguous. proj writes packed per chunk, issued on the Act queue.
    for c in range(NCHUNK):
        ldq = dma
        stq = nc.scalar.dma_start
        hsT = setup.tile([128, P], BF16, tag="hsT")
        ldq(hsT[:], T['hn_T'][0:128, c * P:(c + 1) * P])
        psrc = setup.tile([P, 160], BF16, tag="psrc")
        pdst = setup.tile([P, 160], BF16, tag="pdst")
        pp = ps.tile([P, 128], F32, tag="ps_small")
        nc.tensor.matmul(pp[:], hsT[:], Wsd[:], start=True, stop=True)
        nc.vector.tensor_tensor(
            psrc[:, 0:64], pp[:, 0:64],
            rep[:, ROWS['src_bs'][0]:ROWS['src_bs'][0] + 64], op=OP.add)
        nc.vector.tensor_tensor(
            pdst[:, 0:64], pp[:, 64:128],
            rep[:, ROWS['dst_bs'][0]:ROWS['dst_bs'][0] + 64], op=OP.add)
        hv01 = setup.tile([128, P], BF16, tag="hv01")
        ldq(hv01[:], T['hn_T'][128:256, c * P:(c + 1) * P])
        hv2 = setup.tile([64, P], BF16, tag="hv2")
        ldq(hv2[:], T['hn_T'][256:320, c * P:(c + 1) * P])
        for x in range(3):
            if x == 0:
                lhs, rhs = hv01[0:64, :], Wvsd[:, :]
            elif x == 1:
                lhs, rhs = hv01[64:128, :], WvsdHi[64:128, :]
            else:
                lhs, rhs = hv2[:, :], Wvsd[:, :]
            pv = ps.tile([P, 64], F32, tag="ps_small")
            nc.tensor.matmul(pv[:], lhs, rhs, start=True, stop=True)
            nc.scalar.copy(psrc[:, 64 + 32 * x:96 + 32 * x], pv[:, 0:32])
            nc.scalar.copy(pdst[:, 64 + 32 * x:96 + 32 * x], pv[:, 32:64])
        stq(T['proj_src'][c * P:(c + 1) * P, :], psrc[:])
        stq(T['proj_dst'][c * P:(c + 1) * P, :], pdst[:])

    pm = ps.tile([G, 2 * S_TP], F32, tag="ps_small")
    nc.tensor.matmul(pm[:], tT[:], normWt[:], start=True, stop=True)
    msb = setup.tile([G, 2 * S_TP], F32)
    nc.vector.tensor_tensor(
        msb[:], pm[:], rep[0:G, o_nbt:o_nbt + 2 * S_TP], op=OP.add)
    dma(T['mod_d'][:], msb[:])

    # ---------------- phase B ----------------
    def ln_silu_stage(src_aps, keys, width, pool):
        """Stage-major LayerNorm+SiLU over n independent chains.
        src_aps: list of [P,width] APs; keys: (gname, bname) per chain.
        Returns list of bf16 [P,width] outputs."""
        n = len(src_aps)
        mus = [pool.tile([P, 1], F32, tag="lnmu", name=uid("mu")) for _ in range(n)]
        for i, a in enumerate(src_aps):
            nc.vector.tensor_reduce(mus[i][:], a, axis=AX.X, op=OP.add)
        for i in range(n):
            nc.vector.tensor_scalar_mul(mus[i][:], mus[i][:], 1.0 / width)
        cens = [pool.tile([P, width], F32, tag="lncen", name=uid("cen"))
                for _ in range(n)]
        for i, a in enumerate(src_aps):
            nc.vector.tensor_scalar(cens[i][:], a, mus[i][:, :1], None,
                                    op0=OP.subtract)
        vars_ = [pool.tile([P, 1], F32, tag="lnvar", name=uid("var"))
                 for _ in range(n)]
        for i in range(n):
            sqv = sb.tile([P, width], F32, tag="lnsq", name=uid("sq"))
            nc.scalar.activation(sqv[:], cens[i][:], ACTF.Square,
                                 accum_out=vars_[i][:])
        lnvs = [pool.tile([P, 1], F32, tag="lnv", name=uid("lnv"))
                for _ in range(n)]
        for i in range(n):
            nc.scalar.activation(lnvs[i][:], vars_[i][:], ACTF.Ln,
                                 scale=1.0 / width,
                                 bias=rep[:, ROWS['eps'][0]:ROWS['eps'][0] + 1])
        rstds = [pool.tile([P, 1], F32, tag="rstd", name=uid("rstd"))
                 for _ in range(n)]
        for i in range(n):
            nc.scalar.activation(rstds[i][:], lnvs[i][:], ACTF.Exp, scale=-0.5)
        dests = [pool.tile([P, width], F32, tag="lndst", name=uid("dst"))
                 for _ in range(n)]
        for i in range(n):
            nc.vector.scalar_tensor_tensor(dests[i][:], cens[i][:],
                                           rstds[i][:, :1],
                                           rows_slice(rep, keys[i][0]),
                                           op0=OP.mult, op1=OP.mult)
        for i in range(n):
            nc.vector.tensor_tensor(dests[i][:], dests[i][:],
                                    rows_slice(rep, keys[i][1]), op=OP.add)
        # silu
        es_ = [pool.tile([P, width], F32, tag="silu_e", name=uid("se"))
               for _ in range(n)]
        for i in range(n):
            nc.scalar.activation(es_[i][:], dests[i][:], ACTF.Exp, scale=-1.0)
        for i in range(n):
            nc.vector.tensor_scalar_add(es_[i][:], es_[i][:], 1.0)
        rs_ = [pool.tile([P, width], F32, tag="silu_r", name=uid("sr"))
               for _ in range(n)]
        for i in range(n):
            nc.vector.reciprocal(rs_[i][:], es_[i][:])
        outs = [pool.tile([P, width], BF16, tag="lnout", name=uid("lo"))
                for _ in range(n)]
        for i in range(n):
            nc.vector.tensor_tensor(outs[i][:], dests[i][:], rs_[i][:], op=OP.mult)
        return outs

    g_modA, d_colsA, rvsA, srcfsA, asTs = [], [], [], [], []

    for td in range(NDT):
        E0 = td * DT
        tg = f"t{td}"

        # big per-512 tiles
        s1T = sbT.tile([64, DT], BF16, tag="s1T", name=f"s1T_{td}")
        v1T = sbT.tile([96, DT], BF16, tag="v1T", name=f"v1T_{td}")
        s2T = sbT.tile([64, DT], BF16, tag="s2T", name=f"s2T_{td}")
        v2T = sbT.tile([96, DT], BF16, tag="v2T", name=f"v2T_{td}")
        h2T = {p: sbT.tile([64, DT], BF16, tag=f"h2T{p}", name=f"h2T{p}_{td}")
               for p in ('nf', 'ef')}
        heT = sbT.tile([64, DT], BF16, tag="heT", name=f"heT_{td}")
        dma(heT[:], T['he_T'][0:64, E0:E0 + DT])
        hevT = [sbT.tile([32, DT], BF16, tag=f"hevT{x}", name=f"hevT{x}_{td}")
                for x in range(3)]
        for x in range(3):
            dma(hevT[x][:], T['he_T'][64 + 32 * x:96 + 32 * x, E0:E0 + DT])

        g_mod = []; d_cols = []; rvs = []; srcfs = []; x1sb = []

        # ---- subtile prep: gathers, transposes, RBF, radial ----
        for s in range(NSUB):
            e0 = E0 + s * P
            ef = sbg.tile([P, 5], F32, tag=f"ef{s}", name=uid("ef"))
            dma(ef[:], T['edgef'][e0:e0 + P, :])
            ei = sb.tile([P, 3], I32, tag="ei", name=uid("ei"))
            dma(ei[:], T['edgei'][e0:e0 + P, :])
            d_cols.append(ef[:, 0:1]); rvs.append(ef[:, 1:4]); srcfs.append(ef[:, 4:5])

            g_src = sb.tile([P, 160], BF16, tag="gsrc", name=uid("gs"))
            nc.gpsimd.indirect_dma_start(
                out=g_src[:], out_offset=None, in_=T['proj_src'][:],
                in_offset=IndirectOffsetOnAxis(ap=ei[:, 0:1], axis=0))
            g_dst = sb.tile([P, 160], BF16, tag="gdst", name=uid("gd"))
            nc.gpsimd.indirect_dma_start(
                out=g_dst[:], out_offset=None, in_=T['proj_dst'][:],
                in_offset=IndirectOffsetOnAxis(ap=ei[:, 1:2], axis=0))
            gm = sbg.tile([P, 2 * S_TP], F32, tag=f"gmod{s}", name=uid("gm"))
            nc.gpsimd.indirect_dma_start(
                out=gm[:], out_offset=None, in_=T['mod_d'][:],
                in_offset=IndirectOffsetOnAxis(ap=ei[:, 2:3], axis=0))
            g_mod.append(gm)

            co = s * P
            for (big, src_t, w) in ((s1T, g_src, 64), (v1T, g_src, 96),
                                    (s2T, g_dst, 64), (v2T, g_dst, 96)):
                off = 0 if w == 64 else 64
                tp = psb.tile([w, P], BF16, tag="ps_bf", name=uid("tp"))
                nc.tensor.transpose(tp[:], src_t[:, off:off + w], identB[:])
                nc.scalar.copy(big[:, co:co + P], tp[:])

            # RBF (transposed)
            d_rep = sb.tile([P, P], F32, tag="drep", name=uid("dr"))
            nc.gpsimd.partition_broadcast(d_rep[:], dist_r[:, e0 - 0:e0 + P][:, -P:])
            zT = sb.tile([NB, P], F32, tag="zT", name=uid("zT"))
            nc.vector.scalar_tensor_tensor(zT[:], d_rep[:], rbf_A[:, :1],
                                           ap3(rbf_B, [[0, P]]),
                                           op0=OP.mult, op1=OP.add)
            zsqT = sb.tile([NB, P], F32, tag="zsqT", name=uid("zq"))
            nc.scalar.square(zsqT[:], zT[:])
            esT = sb.tile([NB, P], BF16, tag="esT", name=uid("es"))
            nc.scalar.activation(esT[:], zsqT[:], ACTF.Exp, scale=-0.5)

            # radial first matmul; evac PSUM to SBUF so the 8 LN chains can
            # run stage-major without holding PSUM
            x1 = ps.tile([P, 128], F32, tag="ps_small", name=uid("x1"))
            nc.tensor.matmul(x1[:], esT[:], W1p[:], start=True, stop=True)
            x1s_ = sbx.tile([P, 128], F32, tag="x1sb", name=uid("x1s"))
            nc.scalar.copy(x1s_[:], x1[:])
            x1sb.append(x1s_)

        # ---- radial stage-major: 8 chains = (subtile, nf|ef) ----
        chains = [(s, ri, p) for s in range(NSUB)
                  for ri, p in enumerate(('nf', 'ef'))]
        h1s = ln_silu_stage(
            [x1sb[s][:, 64 * ri:64 * ri + 64] for (s, ri, p) in chains],
            [(p + '_g1', p + '_b1') for (s, ri, p) in chains], 64, sbc)
        x2sb = []
        for i, (s, ri, p) in enumerate(chains):
            h1T_p = psb.tile([64, P], BF16, tag="ps_bf", name=uid("h1t"))
            nc.tensor.transpose(h1T_p[:], h1s[i][:], identB[:])
            h1T = sbc.tile([64, P], BF16, tag="h1T", name=uid("h1T"))
            nc.scalar.copy(h1T[:], h1T_p[:])
            x2 = ps.tile([P, 64], F32, tag="ps_small", name=uid("x2"))
            nc.tensor.matmul(x2[:], h1T[:], (W2nf if p == 'nf' else W2ef)[:],
                             start=True, stop=True)
            x2s_ = sbc.tile([P, 64], F32, tag="x2sb", name=uid("x2s"))
            nc.scalar.copy(x2s_[:], x2[:])
            x2sb.append(x2s_)
        h2s = ln_silu_stage(
            [x2sb[i][:, :] for i in range(8)],
            [(p + '_g2', p + '_b2') for (s, ri, p) in chains], 64, sbc)
        for i, (s, ri, p) in enumerate(chains):
            h2T_p = psb.tile([64, P], BF16, tag="ps_bf", name=uid("h2t"))
            nc.tensor.transpose(h2T_p[:], h2s[i][:], identB[:])
            nc.scalar.copy(h2T[p][:, s * P:s * P + P], h2T_p[:])

        # ---- edge transform (transposed directly) ----
        pes = psm.tile([64, DT], F32, tag="ps_med", name=uid("pes"))
        nc.tensor.matmul(pes[:], etWs[:], heT[:], start=True, stop=True)
        esE = sbT.tile([64, DT], BF16, tag="esE", name=f"esE_{td}")
        nc.vector.tensor_scalar(esE[:], pes[:], bcols[0:64, 3:4], None, op0=OP.add)
        pev = psm.tile([96, DT], F32, tag="ps_med", name=uid("pev"))
        for x in range(3):
            nc.tensor.matmul(pev[32 * x:32 * x + 32, :], etWv[:], hevT[x][:],
                             start=True, stop=True, skip_group_check=True)
        evE = sbT.tile([96, DT], BF16, tag="evE", name=f"evE_{td}")
        nc.scalar.copy(evE[:], pev[:])

        # ---- replicas for the transposed muls ----
        def rep64(src_t, nm):
            r = sbT.tile([128, DT], BF16, tag=nm, name=f"{nm}_{td}")
            nc.vector.tensor_copy(r[0:64, :], src_t[:, :])
            nc.vector.tensor_copy(r[64:128, :], src_t[:, :])
            return r

        def rep32(src_t, x, nm):
            rp = psw.tile([128, DT], F32, tag="psw", name=uid("rp"))
            nc.tensor.matmul(rp[:], R96[:, 128 * x:128 * (x + 1)], src_t[:],
                             start=True, stop=True)
            r = sbT.tile([128, DT], BF16, tag=nm, name=f"{nm}_{td}")
            nc.scalar.copy(r[:], rp[:])
            return r

        s2rep = rep64(s2T, "s2rep")
        esrep = rep64(esE, "esrep")
        v2rep = [rep32(v2T, x, f"v2rep{x}") for x in range(3)]
        evrep = [rep32(evE, x, f"evrep{x}") for x in range(3)]

        # ---- dtp path driver (transposed) ----
        _evac_alt = [0]

        def path(W3t, colbase, nblocks, zreps, UB, uw, outsec):
            """UB: packed one-hot selectors, block b = cols [uw*b, uw*(b+1));
            outsec(xi) -> (psum_tile, base_row). Accumulates over blocks.
            Software-pipelined by one block so PE's reduce-matmul of block b
            doesn't sit in front of block b+1's weight-matmul in program
            order. w-evacs alternate Act/Pool to balance engine load."""
            pend = []
            for b in range(nblocks):
                pw = psw.tile([128, DT], F32, tag="psw", name=uid("pw"))
                nc.tensor.matmul(pw[:], W3t[:, colbase + 128 * b:colbase + 128 * (b + 1)],
                                 (h2T['nf'] if W3t is W3nf else h2T['ef'])[:],
                                 start=True, stop=True)
                single = len(zreps) == 1
                pool_blk = single and b % 4 == 3
                if not single or pool_blk:
                    wsb = sbq.tile([128, DT], BF16, tag="wsb", name=uid("w"))
                    nc.scalar.copy(wsb[:], pw[:])
                qs = []
                for xi, zr in enumerate(zreps):
                    q = sbQ.tile([128, DT], BF16, tag="q", name=uid("q"))
                    if single and not pool_blk:
                        nc.vector.tensor_tensor(q[:], pw[:], zr[:], op=OP.mult)
                    elif pool_blk or xi == 2:
                        nc.gpsimd.tensor_tensor(q[:], wsb[:], zr[:], op=OP.mult)
                    else:
                        nc.vector.tensor_tensor(q[:], wsb[:], zr[:], op=OP.mult)
                    qs.append((xi, q))
                for xi, q in pend:
                    pt, row0 = outsec(xi)
                    nc.tensor.matmul(pt[row0:row0 + uw, :],
                                     UB[:, uw * (b - 1):uw * b], q[:],
                                     start=(b - 1 == 0), stop=False,
                                     skip_group_check=True)
                pend = qs
            for xi, q in pend:
                pt, row0 = outsec(xi)
                nc.tensor.matmul(pt[row0:row0 + uw, :],
                                 UB[:, uw * (nblocks - 1):uw * nblocks], q[:],
                                 start=(nblocks == 1), stop=True,
                                 skip_group_check=True)

        def evac(pt, w, nm):
            t = sbT.tile([w, DT], BF16, tag=nm, name=f"{nm}_{td}")
            nc.scalar.copy(t[:], pt[0:w, :])
            return t

        # dtp1 paths
        p_ss = psp.tile([64, DT], F32, tag="psp", name=uid("pss"))
        path(W3nf, 0, 32, [s2rep], UBss, 64, lambda xi: (p_ss, 0))
        bil1 = evac(p_ss, 64, "bil1")
        p_vs = psp.tile([64, DT], F32, tag="psp", name=uid("pvs"))
        path(W3nf, 6144, 16, [s2rep], UBss, 64, lambda xi: (p_vs, 0))
        bvsR = sbT.tile([96, DT], BF16, tag="bvsR", name=f"bvsR_{td}")
        for x in range(3):
            nc.scalar.copy(bvsR[32 * x:32 * x + 32, :], p_vs[0:32, :])
        p_svA = psp.tile([128, DT], F32, tag="psp", name=uid("psa"))
        p_svB = psp.tile([64, DT], F32, tag="psp", name=uid("psb2"))
        path(W3nf, 4096, 16, v2rep, UBsv, 64,
             lambda xi: (p_svA, 64 * xi) if xi < 2 else (p_svB, 0))
        bsv = []
        for x in range(3):
            t = sbT.tile([64, DT], BF16, tag=f"bsv{x}", name=f"bsv{x}_{td}")
            nc.scalar.copy(t[:], p_svA[64 * x:64 * x + 64, :] if x < 2
                           else p_svB[0:64, :])
            bsv.append(t)
        p_v0 = psp.tile([96, DT], F32, tag="psp", name=uid("pv0"))
        path(W3nf, 8192, 8, v2rep, UB4, 32, lambda xi: (p_v0, 32 * xi))
        bv0 = evac(p_v0, 96, "bv0")
        p_v1 = psp.tile([96, DT], F32, tag="psp", name=uid("pv1"))
        path(W3nf, 9216, 8, v2rep, UB4, 32, lambda xi: (p_v1, 32 * xi))
        # cbuf in two section rotations: P1[s]=cbuf[(s+1)%3], P2[s]=cbuf[(s+2)%3]
        cbufP1 = sbT.tile([96, DT], BF16, tag="cbufP1", name=f"cbufP1_{td}")
        cbufP2 = sbT.tile([96, DT], BF16, tag="cbufP2", name=f"cbufP2_{td}")
        for s_ in range(3):
            nc.scalar.copy(cbufP1[32 * s_:32 * s_ + 32, :],
                           p_v1[32 * ((s_ + 1) % 3):32 * ((s_ + 1) % 3) + 32, :])
            nc.scalar.copy(cbufP2[32 * s_:32 * s_ + 32, :],
                           p_v1[32 * ((s_ + 2) % 3):32 * ((s_ + 2) % 3) + 32, :])

        # ---- epilogue 1: fsT [96, DT], fvT_x [128, DT] (bf16) ----
        fsT = sbT.tile([96, DT], BF16, tag="fsT", name=f"fsT_{td}")
        nc.vector.tensor_tensor(fsT[0:64, :], bil1[:], s1T[:], op=OP.mult)
        t96 = sbT.tile([96, DT], BF16, tag="t96", name=f"t96_{td}")
        nc.vector.tensor_tensor(t96[:], v1T[:], bv0[:], op=OP.mult)
        p_x = psm.tile([32, DT], F32, tag="ps_med", name=uid("px"))
        nc.tensor.matmul(p_x[:], U3[:], t96[:], start=True, stop=True)
        nc.scalar.copy(fsT[64:96, :], p_x[:])
        nc.vector.tensor_scalar(fsT[:], fsT[:], bcols[0:96, 0:1], None, op0=OP.add)

        # cross products, all base-aligned: ta[s] = v1T[s]*cbuf[(s+1)%3],
        # tb[s] = v1T[s]*cbuf[(s+2)%3]; cross_x = ta[(x+1)%3] - tb[(x+2)%3]
        ta_all = sbq.tile([96, DT], BF16, tag="ta", name=uid("ta"))
        nc.vector.tensor_tensor(ta_all[:], v1T[:], cbufP1[:], op=OP.mult)
        tb_all = sbq.tile([96, DT], BF16, tag="tb", name=uid("tb"))
        nc.vector.tensor_tensor(tb_all[:], v1T[:], cbufP2[:], op=OP.mult)
        # rotate tb by +1 section so cross_all[s] = ta_all[s] - tb_rot[s]
        # with cross_all[(x+1)%3] = cross_x
        tb_rot = sbq.tile([96, DT], BF16, tag="tbr", name=uid("tbr"))
        for s_ in range(3):
            nc.vector.tensor_copy(tb_rot[32 * s_:32 * s_ + 32, :],
                                  tb_all[32 * ((s_ + 1) % 3):32 * ((s_ + 1) % 3) + 32, :])
        cross_all = sbq.tile([96, DT], BF16, tag="cra", name=uid("cr"))
        nc.vector.tensor_sub(cross_all[:], ta_all[:], tb_rot[:])
        fvT = []
        for x in range(3):
            fv = sbT.tile([128, DT], BF16, tag=f"fvT{x}", name=f"fvT{x}_{td}")
            nc.vector.tensor_tensor(fv[0:64, :], bsv[x][:], s1T[:], op=OP.mult)
            nc.vector.tensor_tensor(fv[64:96, :], v1T[32 * x:32 * x + 32, :],
                                    bvsR[32 * x:32 * x + 32, :], op=OP.mult)
            yx = (x + 1) % 3
            nc.vector.tensor_copy(fv[96:128, :], cross_all[32 * yx:32 * yx + 32, :])
            fvT.append(fv)

        # ---- node-fusion linear (reads transposed directly) ----
        pns = psm.tile([64, DT], F32, tag="ps_med", name=uid("pns"))
        nc.tensor.matmul(pns[:], ntWs[:], fsT[:], start=True, stop=True)
        nsT = sbT.tile([64, DT], BF16, tag="nsT", name=f"nsT_{td}")
        nc.vector.tensor_scalar(nsT[:], pns[:], bcols[0:64, 2:3], None, op0=OP.add)
        pnv = psm.tile([96, DT], F32, tag="ps_med", name=uid("pnv"))
        for x in range(3):
            nc.tensor.matmul(pnv[32 * x:32 * x + 32, :], ntWv[:], fvT[x][:],
                             start=True, stop=True, skip_group_check=True)
        nvT = sbT.tile([96, DT], BF16, tag="nvT", name=f"nvT_{td}")
        nc.scalar.copy(nvT[:], pnv[:])

        # ---- dtp2 ----
        p_ss2 = psp.tile([64, DT], F32, tag="psp", name=uid("ps2"))
        path(W3ef, 0, 32, [esrep], UBss, 64, lambda xi: (p_ss2, 0))
        bil2 = evac(p_ss2, 64, "bil2")
        p_v02 = psp.tile([96, DT], F32, tag="psp", name=uid("pv2"))
        path(W3ef, 4096, 8, evrep, UB4, 32, lambda xi: (p_v02, 32 * xi))
        bv02 = evac(p_v02, 96, "bv02")

        asT = sbT.tile([96, DT], BF16, tag="asT", name=f"asT_{td}")
        nc.vector.tensor_tensor(asT[0:64, :], bil2[:], nsT[:], op=OP.mult)
        t96b = sbT.tile([96, DT], BF16, tag="t96b", name=f"t96b_{td}")
        nc.vector.tensor_tensor(t96b[:], nvT[:], bv02[:], op=OP.mult)
        p_x2 = psm.tile([32, DT], F32, tag="ps_med", name=uid("px2"))
        nc.tensor.matmul(p_x2[:], U3[:], t96b[:], start=True, stop=True)
        nc.scalar.copy(asT[64:96, :], p_x2[:])
        nc.vector.tensor_scalar(asT[:], asT[:], bcols[0:96, 1:2], None, op0=OP.add)

        # defer adaLN/head/scatter: collect per-512 state
        asTs.append(asT)
        g_modA += g_mod; d_colsA += d_cols; rvsA += rvs; srcfsA += srcfs

    # ---- deferred tail, stage-major across all 8 subtiles ----
    NS8 = NDT * NSUB
    as_l = []
    for k in range(NS8):
        td, s = divmod(k, NSUB)
        as_p = psb.tile([P, S_TP], BF16, tag="ps_bf", name=uid("asp"))
        nc.tensor.transpose(as_p[:], asTs[td][:, s * P:s * P + P],
                            identB[0:96, 0:96])
        as_ = sbc.tile([P, S_TP], F32, tag="as", name=uid("as"))
        nc.scalar.copy(as_[:], as_p[:])
        as_l.append(as_)
    mus = [sbc.tile([P, 1], F32, tag="amu", name=uid("am")) for _ in range(NS8)]
    for k in range(NS8):
        nc.vector.tensor_reduce(mus[k][:], as_l[k][:], axis=AX.X, op=OP.add)
    for k in range(NS8):
        nc.vector.tensor_scalar_mul(mus[k][:], mus[k][:], 1.0 / S_TP)
    cens = [sbc.tile([P, S_TP], F32, tag="acen", name=uid("ac"))
            for _ in range(NS8)]
    for k in range(NS8):
        nc.vector.tensor_scalar(cens[k][:], as_l[k][:], mus[k][:, :1], None,
                                op0=OP.subtract)
    vars_ = [sbc.tile([P, 1], F32, tag="avar", name=uid("av"))
             for _ in range(NS8)]
    for k in range(NS8):
        sqv = sb.tile([P, S_TP], F32, tag="asq", name=uid("aq"))
        nc.scalar.activation(sqv[:], cens[k][:], ACTF.Square,
                             accum_out=vars_[k][:])
    rstds = []
    for k in range(NS8):
        rstds.append(rstd_from_var(vars_[k][:], 1.0 / S_TP, sbc))
    s_ns = [sbc.tile([P, S_TP], BF16, tag="sn", name=uid("sn"))
            for _ in range(NS8)]
    for k in range(NS8):
        nc.vector.scalar_tensor_tensor(s_ns[k][:], cens[k][:],
                                       rstds[k][:, :1],
                                       g_modA[k][:, S_TP:2 * S_TP],
                                       op0=OP.mult, op1=OP.mult)
    for k in range(NS8):
        nc.vector.tensor_tensor(s_ns[k][:], s_ns[k][:], g_modA[k][:, 0:S_TP],
                                op=OP.add)
    hds = []
    for k in range(NS8):
        snT_p = psb.tile([S_TP, P], BF16, tag="ps_bf", name=uid("snp"))
        nc.tensor.transpose(snT_p[:], s_ns[k][:], identB[:])
        snT = sbc.tile([S_TP, P], BF16, tag="snT", name=uid("snT"))
        nc.scalar.copy(snT[:], snT_p[:])
        hd_p = ps.tile([P, 32], F32, tag="ps_small", name=uid("hdp"))
        nc.tensor.matmul(hd_p[:], snT[:], spW1[:], start=True, stop=True)
        hd = sbc.tile([P, 32], F32, tag="hd", name=uid("hd"))
        nc.vector.tensor_tensor(hd[:], hd_p[:], rows_slice(rep, 'sp_b1'),
                                op=OP.add)
        hds.append(hd)
    es_ = [sbc.tile([P, 32], F32, tag="hse", name=uid("he")) for _ in range(NS8)]
    for k in range(NS8):
        nc.scalar.activation(es_[k][:], hds[k][:], ACTF.Exp, scale=-1.0)
    for k in range(NS8):
        nc.vector.tensor_scalar_add(es_[k][:], es_[k][:], 1.0)
    rs_ = [sbc.tile([P, 32], F32, tag="hsr", name=uid("hr")) for _ in range(NS8)]
    for k in range(NS8):
        nc.vector.reciprocal(rs_[k][:], es_[k][:])
    siles = [sbc.tile([P, 32], F32, tag="sile", name=uid("sl"))
             for _ in range(NS8)]
    for k in range(NS8):
        nc.vector.tensor_tensor(siles[k][:], hds[k][:], rs_[k][:], op=OP.mult)
    sws = [sbc.tile([P, 1], F32, tag="swv", name=uid("sv")) for _ in range(NS8)]
    for k in range(NS8):
        swt = sb.tile([P, 32], F32, tag="swt", name=uid("sw"))
        nc.vector.tensor_tensor(swt[:], siles[k][:], rows_slice(rep, 'spW2r'),
                                op=OP.mult)
        nc.vector.tensor_reduce(sws[k][:], swt[:], axis=AX.X, op=OP.add)
        nc.vector.tensor_scalar(sws[k][:], sws[k][:], 32.0 ** -0.5,
                                rep[:, ROWS['sp_b2'][0]:ROWS['sp_b2'][0] + 1],
                                op0=OP.mult, op1=OP.add)
    forces = []
    for k in range(NS8):
        den = sbc.tile([P, 1], F32, tag="den", name=uid("dn"))
        nc.vector.scalar_tensor_tensor(den[:], d_colsA[k], 1.0, d_colsA[k],
                                       op0=OP.add, op1=OP.mult)
        rden = sbc.tile([P, 1], F32, tag="rden", name=uid("rd"))
        nc.vector.reciprocal(rden[:], den[:])
        coef = sbc.tile([P, 1], F32, tag="coef", name=uid("cf"))
        nc.vector.tensor_mul(coef[:], sws[k][:], rden[:])
        force = sbc.tile([P, 3], BF16, tag="force", name=uid("fo"))
        nc.vector.tensor_scalar(force[:], rvsA[k], coef[:, :1], None,
                                op0=OP.mult)
        forces.append(force)

    # scatter: one PSUM accumulator over all 8 subtiles
    acc_p = psp.tile([P, NCHUNK * 3], F32, tag="psp", name=uid("ap"))
    for ch in range(NCHUNK):
        ohs = []
        for k in range(NS8):
            ssh = sbc.tile([P, 1], F32, tag="ssh", name=uid("sh"))
            nc.vector.tensor_scalar_add(ssh[:], srcfsA[k], float(-P * ch))
            oh = sbc.tile([P, P], BF16, tag="oh", name=uid("oh"))
            nc.gpsimd.tensor_scalar(oh[:], iota_f[:], ssh[:, :1], None,
                                    op0=OP.is_equal)
            ohs.append(oh)
        for k in range(NS8):
            nc.tensor.matmul(acc_p[:, 3 * ch:3 * ch + 3], ohs[k][:],
                             forces[k][:],
                             start=(k == 0), stop=(k == NS8 - 1),
                             skip_group_check=True)
    acc_sb = consts.tile([P, NCHUNK * 3], F32)
    nc.scalar.copy(acc_sb[:], acc_p[:])
    dma(T['out'][:], acc_sb[:])


# ======================= host side =======================

def _bf16(a):
    return np.asarray(a, np.float32).astype(mybir.dt.np(BF16))


def host_prep(inp):
    inp = {k: np.asarray(v) for k, v in inp.items()}
    src = inp['edge_index'][0].astype(np.int32)
    dst = inp['edge_index'][1].astype(np.int32)
    perm = np.argsort(src, kind='stable')
    src, dst = src[perm], dst[perm]
    gid = inp['batch'].astype(np.int32)[src]
    h_edge = inp['h_edge'][perm]
    dist = inp['distance'][perm].astype(np.float32)
    rvec = inp['relative_vec'][perm].astype(np.float32)

    rows = np.zeros(RWID, np.float32)

    def setr(name, val):
        off, w = ROWS[name]
        rows[off:off + w] = val
    for p in ('nf', 'ef'):
        for q in ('g1', 'b1', 'g2', 'b2'):
            setr(f'{p}_{q}', inp[f'{p}_{q}'])
    setr('src_bs', inp['src_bs']); setr('dst_bs', inp['dst_bs'])
    setr('sp_b1', inp['sp_b1']); setr('spW2r', inp['sp_W2'][:, 0])
    rows[ROWS['sp_b2'][0]] = inp['sp_b2'][0]
    rows[ROWS['eps'][0]] = 1e-5
    setr('normbt', inp['norm_bt'][:2 * S_TP])

    std = np.asarray(inp['rbf_std'], np.float32)
    mean = np.asarray(inp['rbf_mean'], np.float32)
    rbf_w = float(np.asarray(inp['rbf_w'])); rbf_b = float(np.asarray(inp['rbf_b']))
    A = (rbf_w / (CUTOFF * std)).reshape(-1, 1).astype(np.float32)
    Bc = ((rbf_b - mean) / std).reshape(-1, 1).astype(np.float32)
    cnorm = (1.0 / (np.sqrt(2 * np.pi) * std)).reshape(-1, 1).astype(np.float32)
    W1p = np.concatenate([inp['nf_W1'], inp['ef_W1']], axis=1) * cnorm

    def fold_w3(W3):
        W3 = np.asarray(W3, np.float32).copy()
        W3[:, 0:4096] *= 64.0 ** -0.5
        W3[:, 4096:6144] *= 32.0 ** -0.5
        W3[:, 6144:8192] *= 64.0 ** -0.5
        W3[:, 8192:9216] *= 96.0 ** -0.5
        W3[:, 9216:10240] *= 64.0 ** -0.5
        return W3
    W3nf = fold_w3(inp['nf_W3'])
    W3ef_f = fold_w3(inp['ef_W3'])
    W3ef = np.concatenate([W3ef_f[:, :4096], W3ef_f[:, 8192:9216]], axis=1)

    Wsd = np.concatenate([inp['src_Ws'], inp['dst_Ws']], axis=1) * 128.0 ** -0.5
    Wvsd = np.concatenate([inp['src_Wv'], inp['dst_Wv']], axis=1) * 64.0 ** -0.5

    pp = np.arange(128)
    # UBss: block b (of 32) -> row 2b + p//64, packed as cols [64b, 64b+64)
    UBss = np.zeros((128, 2048), np.float32)
    for b in range(32):
        UBss[pp, 64 * b + np.minimum(2 * b + pp // 64, 63)] = (2 * b + pp // 64 < 64)
    # UBsv: block b (of 16) -> row 4b + p//32, cols [64b, 64b+64)
    UBsv = np.zeros((128, 1024), np.float32)
    for b in range(16):
        UBsv[pp, 64 * b + 4 * b + pp // 32] = 1.0
    # UB4: block b (of 8) -> row 4b + p//32, cols [32b, 32b+32)
    UB4 = np.zeros((128, 256), np.float32)
    for b in range(8):
        UB4[pp, 32 * b + 4 * b + pp // 32] = 1.0
    U3 = (np.arange(96)[:, None] % 32 == np.arange(32)[None, :]).astype(np.float32)
    R96 = np.zeros((96, 384), np.float32)
    for x in range(3):
        R96[:, 128 * x:128 * (x + 1)] = (
            np.arange(96)[:, None] == 32 * x + np.arange(128)[None, :] % 32)
    bcols = np.zeros((96, 4), np.float32)
    bcols[:, 0] = inp['nf_bias']; bcols[:, 1] = inp['ef_bias']
    bcols[:64, 2] = inp['nt_bs']; bcols[:64, 3] = inp['et_bs']

    # device row order: scalars then x-grouped vector components
    hnT = np.ascontiguousarray(inp['h_node'].T)          # [320, N]
    hnT = np.concatenate([hnT[0:128], hnT[128::3], hnT[129::3], hnT[130::3]], 0)
    shared = dict(
        hn_T=_bf16(hnT),
        t_T=_bf16(np.ascontiguousarray(inp['t'].T)),
        normWt=_bf16(np.ascontiguousarray(inp['norm_Wt'][:, :2 * S_TP])),
        rows=np.ascontiguousarray(rows.reshape(1, -1), np.float32),
        rbf_A=A, rbf_B=Bc,
        W1p=_bf16(W1p),
        nf_W2=_bf16(inp['nf_W2']), ef_W2=_bf16(inp['ef_W2']),
        W3nf=_bf16(W3nf), W3ef=_bf16(W3ef),
        Wsd=_bf16(Wsd), Wvsd=_bf16(Wvsd),
        nt_Ws=_bf16(inp['nt_Ws'] * 96.0 ** -0.5),
        nt_Wv=_bf16(inp['nt_Wv'] * 128.0 ** -0.5),
        et_Ws=_bf16(inp['et_Ws'] * 64.0 ** -0.5),
        et_Wv=_bf16(inp['et_Wv'] * 32.0 ** -0.5),
        sp_W1=_bf16(inp['sp_W1'] * 96.0 ** -0.5),
        UBss=_bf16(UBss), UBsv=_bf16(UBsv), UB4=_bf16(UB4),
        U3=_bf16(U3), R96=_bf16(R96),
        bcols=bcols,
    )

    in_maps = []
    for c in range(NC_CORES):
        sl = slice(c * EC, (c + 1) * EC)
        m = dict(shared)
        heT_ = np.ascontiguousarray(h_edge[sl].T)        # [160, EC]
        heT_ = np.concatenate([heT_[0:64], heT_[64::3], heT_[65::3], heT_[66::3]], 0)
        m['he_T'] = _bf16(heT_)
        m['dist_r'] = np.ascontiguousarray(dist[sl].reshape(1, -1))
        m['edgef'] = np.ascontiguousarray(np.concatenate(
            [dist[sl, None], rvec[sl], src[sl, None].astype(np.float32)],
            axis=1), np.float32)
        m['edgei'] = np.ascontiguousarray(np.stack(
            [src[sl], dst[sl], gid[sl]], axis=1).astype(np.int32))
        in_maps.append(m)
    return in_maps


_CACHED_NC = None


def kernel(**inputs):
    global _CACHED_NC
    from concourse.bass_utils import run_bass_kernel_spmd
    if _CACHED_NC is None:
        _CACHED_NC = build_nc()
    in_maps = host_prep(inputs)
    res = run_bass_kernel_spmd(_CACHED_NC, in_maps, list(range(NC_CORES)))
    out = np.zeros((128, NCHUNK, 3), np.float32)
    for r in res.results:
        out += r['out'].reshape(128, NCHUNK, 3)
    return np.ascontiguousarray(out.transpose(1, 0, 2).reshape(N, 3))


# revision 59
# speedup vs baseline: 1.1161x; 1.0231x over previous
"""Bass/Trainium2 kernel for nn_EquivariantPosUpdate — 8-core edge-parallel.

v2: transposed depthwise-TP. Per core, 1024 edges in 2 dtp-tiles of DT=512
(4 subtiles of 128 for gather/RBF/radial/adaLN work that needs edges on
partitions).

Key ideas vs v1:
  - bf16 everywhere PE touches (4x matmul, 2x DVE) with scales folded on host
  - TP weights computed TRANSPOSED: w^T[(u,v), e] = matmul(W3-block, h2T),
    so the v-contraction becomes a one-hot PE matmul (PSUM row writes)
    instead of a DVE TensorReduce — the v1 bottleneck (238us of DVE reduces)
    moves to the 4x-faster, underutilized PE
  - all epilogues run in [feat, e] layout; nt/et linears consume them
    directly as lhsT (no fs/fv transposes)
  - one activation table (exp/ln/square/copy): rstd = exp(-0.5*ln(var)),
    silu via exp + fast-reciprocal on DVE
"""
import sys
sys.path.insert(0, '/opt/trn_rl_repo')
import numpy as np
from contextlib import ExitStack

import concourse.bass as bass
import concourse.bacc as bacc
import concourse.mybir as mybir
import concourse.tile as tile
from concourse.bass import AP, IndirectOffsetOnAxis
from concourse.masks import make_identity

F32 = mybir.dt.float32
BF16 = mybir.dt.bfloat16
I32 = mybir.dt.int32
AX = mybir.AxisListType
OP = mybir.AluOpType
ACTF = mybir.ActivationFunctionType

N, E, G, NB = 2048, 8192, 64, 128
NC_CORES = 8
EC = E // NC_CORES          # 1024
P = 128
DT = 512                    # dtp tile (edges on free axis)
NDT = EC // DT              # 2
NSUB = DT // P              # 4
M0, M1 = 64, 32
S_TP = 96
CUTOFF = 5.0
NCHUNK = N // P             # 16

ROWS = {}
_off = 0
for _n, _w in [('nf_g1', 64), ('nf_b1', 64), ('nf_g2', 64), ('nf_b2', 64),
               ('ef_g1', 64), ('ef_b1', 64), ('ef_g2', 64), ('ef_b2', 64),
               ('src_bs', 64), ('dst_bs', 64),
               ('sp_b1', 32), ('spW2r', 32),
               ('sp_b2', 1), ('eps', 1), ('normbt', 192)]:
    ROWS[_n] = (_off, _w)
    _off += _w
RWID = _off


def rows_slice(rep, name):
    off, w = ROWS[name]
    return rep[:, off:off + w]


def ap3(t, dims, offset=0):
    base = t[:, :] if not isinstance(t, AP) else t
    ap = AP(base.tensor, base.offset + offset, [base.ap[0]] + [list(d) for d in dims])
    return ap


def build_nc():
    nc = bacc.Bacc("TRN2", target_bir_lowering=False, debug=False,
                   num_devices=NC_CORES)
    T = {}

    def din(name, shape, dtype=F32):
        T[name] = nc.dram_tensor(name, shape, dtype, kind="ExternalInput")
        return T[name]

    din('hn_T', [320, N], BF16); din('he_T', [160, EC], BF16)
    din('dist_r', [1, EC])
    din('edgef', [EC, 5]); din('edgei', [EC, 3], I32)
    din('t_T', [128, G], BF16); din('normWt', [128, 2 * S_TP], BF16)
    din('rows', [1, RWID])
    din('rbf_A', [NB, 1]); din('rbf_B', [NB, 1])
    din('W1p', [NB, 128], BF16)
    din('nf_W2', [64, 64], BF16); din('ef_W2', [64, 64], BF16)
    din('W3nf', [64, 10240], BF16); din('W3ef', [64, 5120], BF16)
    din('Wsd', [128, 128], BF16); din('Wvsd', [64, 64], BF16)
    din('nt_Ws', [S_TP, 64], BF16); din('nt_Wv', [128, 32], BF16)
    din('et_Ws', [64, 64], BF16); din('et_Wv', [32, 32], BF16)
    din('sp_W1', [S_TP, 32], BF16)
    din('UBss', [128, 2048], BF16); din('UBsv', [128, 1024], BF16)
    din('UB4', [128, 256], BF16)
    din('U3', [96, 32], BF16); din('R96', [96, 384], BF16)
    din('bcols', [96, 4])
    out = nc.dram_tensor('out', [128, NCHUNK * 3], F32, kind="ExternalOutput")
    T['out'] = out
    T['proj_src'] = nc.dram_tensor('proj_src', [N, 160], BF16)
    T['proj_dst'] = nc.dram_tensor('proj_dst', [N, 160], BF16)
    T['mod_d'] = nc.dram_tensor('mod_d', [G, 2 * S_TP], F32)

    with tile.TileContext(nc) as tc:
        with ExitStack() as ctx:
            _build(ctx, tc, nc, T)
    nc.compile()
    return nc


def _build(ctx, tc, nc, T):
    consts = ctx.enter_context(tc.tile_pool(name="consts", bufs=1))
    setup = ctx.enter_context(tc.tile_pool(name="setup", bufs=2))
    sb = ctx.enter_context(tc.tile_pool(name="sb", bufs=3))
    sbq = ctx.enter_context(tc.tile_pool(name="sbq", bufs=3))
    sbQ = ctx.enter_context(tc.tile_pool(name="sbQ", bufs=6))
    sbg = ctx.enter_context(tc.tile_pool(name="sbg", bufs=2))
    sbT = ctx.enter_context(tc.tile_pool(name="sbT", bufs=2))
    sbc = ctx.enter_context(tc.tile_pool(name="sbc", bufs=8))
    sbx = ctx.enter_context(tc.tile_pool(name="sbx", bufs=4))
    ps = ctx.enter_context(tc.tile_pool(name="ps", bufs=2, space="PSUM"))
    psm = ctx.enter_context(tc.tile_pool(name="psm", bufs=1, space="PSUM"))
    psb = ctx.enter_context(tc.tile_pool(name="psb", bufs=1, space="PSUM"))
    psw = ctx.enter_context(tc.tile_pool(name="psw", bufs=2, space="PSUM"))
    psp = ctx.enter_context(tc.tile_pool(name="psp", bufs=2, space="PSUM"))
    dma = nc.sync.dma_start

    def load(name, pool=consts):
        t = pool.tile(T[name].shape, T[name].dtype, tag="ld_" + name,
                      name="ld_" + name)
        dma(t[:], T[name][:])
        return t

    # ---------------- setup ----------------
    # pin one act table (exp/ln/square/copy) so the table-load pass never
    # needs to swap between exp-only and ln-only sets
    PIN_ACT_TABLE = True
    if PIN_ACT_TABLE:
        from concourse.hw_specs import get_activation_tables
        _tabs = list(get_activation_tables(nc.m.arch))
        _tid = _tabs.index('natural_log_exp_and_others')
        nc.scalar.add_instruction(mybir.InstLoadActFuncSet(
            name=nc.get_next_instruction_name(), ins=[], outs=[],
            act_func_set_id=_tid))
    ident = consts.tile([P, P], F32)
    make_identity(nc, ident[:])
    identB = consts.tile([P, P], BF16)
    nc.vector.tensor_copy(identB[:], ident[:])
    iota_i = consts.tile([P, P], I32)
    nc.gpsimd.iota(iota_i[:], pattern=[[1, P]], base=0, channel_multiplier=0)
    iota_f = consts.tile([P, P], F32)
    nc.vector.tensor_copy(iota_f[:], iota_i[:])

    rows1 = consts.tile([1, RWID], F32)
    dma(rows1[:], T['rows'][:])
    o_nbt = ROWS['normbt'][0]
    nc.vector.tensor_scalar_add(rows1[:, o_nbt + S_TP:o_nbt + 2 * S_TP],
                                rows1[:, o_nbt + S_TP:o_nbt + 2 * S_TP], 1.0)
    rep = consts.tile([P, RWID], F32)
    nc.gpsimd.partition_broadcast(rep[:], rows1[:])

    Wsd = load('Wsd'); Wvsd = load('Wvsd')
    WvsdHi = consts.tile([128, 64], BF16, tag="wvhi")
    nc.vector.tensor_copy(WvsdHi[64:128, :], Wvsd[:, :])
    rbf_A = load('rbf_A'); rbf_B = load('rbf_B')
    dist_r = load('dist_r')
    W1p = load('W1p')
    W2nf = load('nf_W2'); W2ef = load('ef_W2')
    ntWs = load('nt_Ws'); ntWv = load('nt_Wv')
    etWs = load('et_Ws'); etWv = load('et_Wv')
    spW1 = load('sp_W1')
    normWt = load('normWt'); tT = load('t_T')
    U3 = load('U3'); R96 = load('R96')
    bcols = load('bcols')
    UBss = load('UBss'); UBsv = load('UBsv'); UB4 = load('UB4')
    W3nf = consts.tile(T['W3nf'].shape, BF16, tag="ld_W3nf", name="ld_W3nf")
    nc.scalar.dma_start(W3nf[:], T['W3nf'][:])
    W3ef = consts.tile(T['W3ef'].shape, BF16, tag="ld_W3ef", name="ld_W3ef")
    nc.scalar.dma_start(W3ef[:], T['W3ef'][:])

    _uid = [0]

    def uid(tag):
        _uid[0] += 1
        return f"{tag}_{_uid[0]}"

    def silu(dst, src_ap, width, pool):
        e_ = pool.tile([P, width], F32, tag="silu_e", name=uid("se"))
        nc.scalar.activation(e_[:], src_ap, ACTF.Exp, scale=-1.0)
        d_ = pool.tile([P, width], F32, tag="silu_d", name=uid("sd"))
        nc.vector.tensor_scalar_add(d_[:], e_[:], 1.0)
        r_ = pool.tile([P, width], F32, tag="silu_r", name=uid("sr"))
        nc.vector.reciprocal_approx_fast(out=r_[:], in_=d_[:])
        nc.vector.tensor_tensor(dst, src_ap, r_[:], op=OP.mult)

    def rstd_from_var(var_ap, inv_width, pool):
        lnv = pool.tile([P, 1], F32, tag="lnv", name=uid("lnv"))
        nc.scalar.activation(lnv[:], var_ap, ACTF.Ln, scale=inv_width,
                             bias=rep[:, ROWS['eps'][0]:ROWS['eps'][0] + 1])
        rstd = pool.tile([P, 1], F32, tag="rstd", name=uid("rstd"))
        nc.scalar.activation(rstd[:], lnv[:], ACTF.Exp, scale=-0.5)
        return rstd

    # ---------------- phase A: node projections (bf16) ----------------
    # hn_T host row order: [s(128) | x0(64) | x1(64) | x2(64)] so each load
    # is contiguous. proj writes packed per chunk, issued on the Act queue.
    for c in range(NCHUNK):
        ldq = dma
        stq = nc.scalar.dma_start
        hsT = setup.tile([128, P], BF16, tag="hsT")
        ldq(hsT[:], T['hn_T'][0:128, c * P:(c + 1) * P])
        psrc = setup.tile([P, 160], BF16, tag="psrc")
        pdst = setup.tile([P, 160], BF16, tag="pdst")
        pp = ps.tile([P, 128], F32, tag="ps_small")
        nc.tensor.matmul(pp[:], hsT[:], Wsd[:], start=True, stop=True)
        nc.vector.tensor_tensor(
            psrc[:, 0:64], pp[:, 0:64],
            rep[:, ROWS['src_bs'][0]:ROWS['src_bs'][0] + 64], op=OP.add)
        nc.vector.tensor_tensor(
            pdst[:, 0:64], pp[:, 64:128],
            rep[:, ROWS['dst_bs'][0]:ROWS['dst_bs'][0] + 64], op=OP.add)
        hv01 = setup.tile([128, P], BF16, tag="hv01")
        ldq(hv01[:], T['hn_T'][128:256, c * P:(c + 1) * P])
        hv2 = setup.tile([64, P], BF16, tag="hv2")
        ldq(hv2[:], T['hn_T'][256:320, c * P:(c + 1) * P])
        for x in range(3):
            if x == 0:
                lhs, rhs = hv01[0:64, :], Wvsd[:, :]
            elif x == 1:
                lhs, rhs = hv01[64:128, :], WvsdHi[64:128, :]
            else:
                lhs, rhs = hv2[:, :], Wvsd[:, :]
            pv = ps.tile([P, 64], F32, tag="ps_small")
            nc.tensor.matmul(pv[:], lhs, rhs, start=True, stop=True)
            nc.scalar.copy(psrc[:, 64 + 32 * x:96 + 32 * x], pv[:, 0:32])
            nc.scalar.copy(pdst[:, 64 + 32 * x:96 + 32 * x], pv[:, 32:64])
        wq = nc.scalar.dma_start if c % 2 == 0 else dma
        wq(T['proj_src'][c * P:(c + 1) * P, :], psrc[:])
        wq(T['proj_dst'][c * P:(c + 1) * P, :], pdst[:])

    pm = ps.tile([G, 2 * S_TP], F32, tag="ps_small")
    nc.tensor.matmul(pm[:], tT[:], normWt[:], start=True, stop=True)
    msb = setup.tile([G, 2 * S_TP], F32)
    nc.vector.tensor_tensor(
        msb[:], pm[:], rep[0:G, o_nbt:o_nbt + 2 * S_TP], op=OP.add)
    dma(T['mod_d'][:], msb[:])

    # ---------------- phase B ----------------
    def ln_silu_stage(src_aps, keys, width, pool):
        """Stage-major LayerNorm+SiLU over n independent chains.
        src_aps: list of [P,width] APs; keys: (gname, bname) per chain.
        Returns list of bf16 [P,width] outputs."""
        n = len(src_aps)
        mus = [pool.tile([P, 1], F32, tag="lnmu", name=uid("mu")) for _ in range(n)]
        for i, a in enumerate(src_aps):
            nc.vector.tensor_reduce(mus[i][:], a, axis=AX.X, op=OP.add)
        for i in range(n):
            nc.vector.tensor_scalar_mul(mus[i][:], mus[i][:], 1.0 / width)
        cens = [pool.tile([P, width], F32, tag="lncen", name=uid("cen"))
                for _ in range(n)]
        for i, a in enumerate(src_aps):
            nc.vector.tensor_scalar(cens[i][:], a, mus[i][:, :1], None,
                                    op0=OP.subtract)
        vars_ = [pool.tile([P, 1], F32, tag="lnvar", name=uid("var"))
                 for _ in range(n)]
        for i in range(n):
            sqv = sb.tile([P, width], F32, tag="lnsq", name=uid("sq"))
            nc.scalar.activation(sqv[:], cens[i][:], ACTF.Square,
                                 accum_out=vars_[i][:])
        lnvs = [pool.tile([P, 1], F32, tag="lnv", name=uid("lnv"))
                for _ in range(n)]
        for i in range(n):
            nc.scalar.activation(lnvs[i][:], vars_[i][:], ACTF.Ln,
                                 scale=1.0 / width,
                                 bias=rep[:, ROWS['eps'][0]:ROWS['eps'][0] + 1])
        rstds = [pool.tile([P, 1], F32, tag="rstd", name=uid("rstd"))
                 for _ in range(n)]
        for i in range(n):
            nc.scalar.activation(rstds[i][:], lnvs[i][:], ACTF.Exp, scale=-0.5)
        dests = [pool.tile([P, width], F32, tag="lndst", name=uid("dst"))
                 for _ in range(n)]
        for i in range(n):
            nc.vector.scalar_tensor_tensor(dests[i][:], cens[i][:],
                                           rstds[i][:, :1],
                                           rows_slice(rep, keys[i][0]),
                                           op0=OP.mult, op1=OP.mult)
        for i in range(n):
            nc.vector.tensor_tensor(dests[i][:], dests[i][:],
                                    rows_slice(rep, keys[i][1]), op=OP.add)
        # silu
        es_ = [pool.tile([P, width], F32, tag="silu_e", name=uid("se"))
               for _ in range(n)]
        for i in range(n):
            nc.scalar.activation(es_[i][:], dests[i][:], ACTF.Exp, scale=-1.0)
        for i in range(n):
            nc.vector.tensor_scalar_add(es_[i][:], es_[i][:], 1.0)
        rs_ = [pool.tile([P, width], F32, tag="silu_r", name=uid("sr"))
               for _ in range(n)]
        for i in range(n):
            nc.vector.reciprocal(rs_[i][:], es_[i][:])
        outs = [pool.tile([P, width], BF16, tag="lnout", name=uid("lo"))
                for _ in range(n)]
        for i in range(n):
            nc.vector.tensor_tensor(outs[i][:], dests[i][:], rs_[i][:], op=OP.mult)
        return outs

    g_modA, d_colsA, rvsA, srcfsA, asTs = [], [], [], [], []

    for td in range(NDT):
        E0 = td * DT
        tg = f"t{td}"

        # big per-512 tiles
        s1T = sbT.tile([64, DT], BF16, tag="s1T", name=f"s1T_{td}")
        v1T = sbT.tile([96, DT], BF16, tag="v1T", name=f"v1T_{td}")
        s2T = sbT.tile([64, DT], BF16, tag="s2T", name=f"s2T_{td}")
        v2T = sbT.tile([96, DT], BF16, tag="v2T", name=f"v2T_{td}")
        h2T = {p: sbT.tile([64, DT], BF16, tag=f"h2T{p}", name=f"h2T{p}_{td}")
               for p in ('nf', 'ef')}
        heT = sbT.tile([64, DT], BF16, tag="heT", name=f"heT_{td}")
        dma(heT[:], T['he_T'][0:64, E0:E0 + DT])
        hevT = [sbT.tile([32, DT], BF16, tag=f"hevT{x}", name=f"hevT{x}_{td}")
                for x in range(3)]
        for x in range(3):
            dma(hevT[x][:], T['he_T'][64 + 32 * x:96 + 32 * x, E0:E0 + DT])

        g_mod = []; d_cols = []; rvs = []; srcfs = []; x1sb = []

        # ---- subtile prep: gathers, transposes, RBF, radial ----
        for s in range(NSUB):
            e0 = E0 + s * P
            ef = sbg.tile([P, 5], F32, tag=f"ef{s}", name=uid("ef"))
            dma(ef[:], T['edgef'][e0:e0 + P, :])
            ei = sb.tile([P, 3], I32, tag="ei", name=uid("ei"))
            dma(ei[:], T['edgei'][e0:e0 + P, :])
            d_cols.append(ef[:, 0:1]); rvs.append(ef[:, 1:4]); srcfs.append(ef[:, 4:5])

            g_src = sb.tile([P, 160], BF16, tag="gsrc", name=uid("gs"))
            nc.gpsimd.indirect_dma_start(
                out=g_src[:], out_offset=None, in_=T['proj_src'][:],
                in_offset=IndirectOffsetOnAxis(ap=ei[:, 0:1], axis=0))
            g_dst = sb.tile([P, 160], BF16, tag="gdst", name=uid("gd"))
            nc.gpsimd.indirect_dma_start(
                out=g_dst[:], out_offset=None, in_=T['proj_dst'][:],
                in_offset=IndirectOffsetOnAxis(ap=ei[:, 1:2], axis=0))
            gm = sbg.tile([P, 2 * S_TP], F32, tag=f"gmod{s}", name=uid("gm"))
            nc.gpsimd.indirect_dma_start(
                out=gm[:], out_offset=None, in_=T['mod_d'][:],
                in_offset=IndirectOffsetOnAxis(ap=ei[:, 2:3], axis=0))
            g_mod.append(gm)

            co = s * P
            for (big, src_t, w) in ((s1T, g_src, 64), (v1T, g_src, 96),
                                    (s2T, g_dst, 64), (v2T, g_dst, 96)):
                off = 0 if w == 64 else 64
                tp = psb.tile([w, P], BF16, tag="ps_bf", name=uid("tp"))
                nc.tensor.transpose(tp[:], src_t[:, off:off + w], identB[:])
                nc.scalar.copy(big[:, co:co + P], tp[:])

            # RBF (transposed)
            d_rep = sb.tile([P, P], F32, tag="drep", name=uid("dr"))
            nc.gpsimd.partition_broadcast(d_rep[:], dist_r[:, e0 - 0:e0 + P][:, -P:])
            zT = sb.tile([NB, P], F32, tag="zT", name=uid("zT"))
            nc.vector.scalar_tensor_tensor(zT[:], d_rep[:], rbf_A[:, :1],
                                           ap3(rbf_B, [[0, P]]),
                                           op0=OP.mult, op1=OP.add)
            zsqT = sb.tile([NB, P], F32, tag="zsqT", name=uid("zq"))
            nc.scalar.square(zsqT[:], zT[:])
            esT = sb.tile([NB, P], BF16, tag="esT", name=uid("es"))
            nc.scalar.activation(esT[:], zsqT[:], ACTF.Exp, scale=-0.5)

            # radial first matmul; evac PSUM to SBUF so the 8 LN chains can
            # run stage-major without holding PSUM
            x1 = ps.tile([P, 128], F32, tag="ps_small", name=uid("x1"))
            nc.tensor.matmul(x1[:], esT[:], W1p[:], start=True, stop=True)
            x1s_ = sbx.tile([P, 128], F32, tag="x1sb", name=uid("x1s"))
            nc.scalar.copy(x1s_[:], x1[:])
            x1sb.append(x1s_)

        # ---- radial stage-major: 8 chains = (subtile, nf|ef) ----
        chains = [(s, ri, p) for s in range(NSUB)
                  for ri, p in enumerate(('nf', 'ef'))]
        h1s = ln_silu_stage(
            [x1sb[s][:, 64 * ri:64 * ri + 64] for (s, ri, p) in chains],
            [(p + '_g1', p + '_b1') for (s, ri, p) in chains], 64, sbc)
        x2sb = []
        for i, (s, ri, p) in enumerate(chains):
            h1T_p = psb.tile([64, P], BF16, tag="ps_bf", name=uid("h1t"))
            nc.tensor.transpose(h1T_p[:], h1s[i][:], identB[:])
            h1T = sbc.tile([64, P], BF16, tag="h1T", name=uid("h1T"))
            nc.scalar.copy(h1T[:], h1T_p[:])
            x2 = ps.tile([P, 64], F32, tag="ps_small", name=uid("x2"))
            nc.tensor.matmul(x2[:], h1T[:], (W2nf if p == 'nf' else W2ef)[:],
                             start=True, stop=True)
            x2s_ = sbc.tile([P, 64], F32, tag="x2sb", name=uid("x2s"))
            nc.scalar.copy(x2s_[:], x2[:])
            x2sb.append(x2s_)
        h2s = ln_silu_stage(
            [x2sb[i][:, :] for i in range(8)],
            [(p + '_g2', p + '_b2') for (s, ri, p) in chains], 64, sbc)
        for i, (s, ri, p) in enumerate(chains):
            h2T_p = psb.tile([64, P], BF16, tag="ps_bf", name=uid("h2t"))
            nc.tensor.transpose(h2T_p[:], h2s[i][:], identB[:])
            nc.scalar.copy(h2T[p][:, s * P:s * P + P], h2T_p[:])

        # ---- edge transform (transposed directly) ----
        pes = psm.tile([64, DT], F32, tag="ps_med", name=uid("pes"))
        nc.tensor.matmul(pes[:], etWs[:], heT[:], start=True, stop=True)
        esE = sbT.tile([64, DT], BF16, tag="esE", name=f"esE_{td}")
        nc.vector.tensor_scalar(esE[:], pes[:], bcols[0:64, 3:4], None, op0=OP.add)
        pev = psm.tile([96, DT], F32, tag="ps_med", name=uid("pev"))
        for x in range(3):
            nc.tensor.matmul(pev[32 * x:32 * x + 32, :], etWv[:], hevT[x][:],
                             start=True, stop=True, skip_group_check=True)
        evE = sbT.tile([96, DT], BF16, tag="evE", name=f"evE_{td}")
        nc.scalar.copy(evE[:], pev[:])

        # ---- replicas for the transposed muls ----
        def rep64(src_t, nm):
            r = sbT.tile([128, DT], BF16, tag=nm, name=f"{nm}_{td}")
            nc.vector.tensor_copy(r[0:64, :], src_t[:, :])
            nc.vector.tensor_copy(r[64:128, :], src_t[:, :])
            return r

        def rep32(src_t, x, nm):
            rp = psw.tile([128, DT], F32, tag="psw", name=uid("rp"))
            nc.tensor.matmul(rp[:], R96[:, 128 * x:128 * (x + 1)], src_t[:],
                             start=True, stop=True)
            r = sbT.tile([128, DT], BF16, tag=nm, name=f"{nm}_{td}")
            nc.scalar.copy(r[:], rp[:])
            return r

        s2rep = rep64(s2T, "s2rep")
        esrep = rep64(esE, "esrep")
        v2rep = [rep32(v2T, x, f"v2rep{x}") for x in range(3)]
        evrep = [rep32(evE, x, f"evrep{x}") for x in range(3)]

        # ---- dtp path driver (transposed) ----
        _evac_alt = [0]

        def path(W3t, colbase, nblocks, zreps, UB, uw, outsec):
            """UB: packed one-hot selectors, block b = cols [uw*b, uw*(b+1));
            outsec(xi) -> (psum_tile, base_row). Accumulates over blocks.
            Software-pipelined by one block so PE's reduce-matmul of block b
            doesn't sit in front of block b+1's weight-matmul in program
            order. w-evacs alternate Act/Pool to balance engine load."""
            pend = []
            for b in range(nblocks):
                pw = psw.tile([128, DT], F32, tag="psw", name=uid("pw"))
                nc.tensor.matmul(pw[:], W3t[:, colbase + 128 * b:colbase + 128 * (b + 1)],
                                 (h2T['nf'] if W3t is W3nf else h2T['ef'])[:],
                                 start=True, stop=True)
                single = len(zreps) == 1
                pool_blk = single and b % 4 == 3
                if not single or pool_blk:
                    wsb = sbq.tile([128, DT], BF16, tag="wsb", name=uid("w"))
                    nc.scalar.copy(wsb[:], pw[:])
                qs = []
                for xi, zr in enumerate(zreps):
                    q = sbQ.tile([128, DT], BF16, tag="q", name=uid("q"))
                    if single and not pool_blk:
                        nc.vector.tensor_tensor(q[:], pw[:], zr[:], op=OP.mult)
                    elif pool_blk or xi == 2:
                        nc.gpsimd.tensor_tensor(q[:], wsb[:], zr[:], op=OP.mult)
                    else:
                        nc.vector.tensor_tensor(q[:], wsb[:], zr[:], op=OP.mult)
                    qs.append((xi, q))
                for xi, q in pend:
                    pt, row0 = outsec(xi)
                    nc.tensor.matmul(pt[row0:row0 + uw, :],
                                     UB[:, uw * (b - 1):uw * b], q[:],
                                     start=(b - 1 == 0), stop=False,
                                     skip_group_check=True)
                pend = qs
            for xi, q in pend:
                pt, row0 = outsec(xi)
                nc.tensor.matmul(pt[row0:row0 + uw, :],
                                 UB[:, uw * (nblocks - 1):uw * nblocks], q[:],
                                 start=(nblocks == 1), stop=True,
                                 skip_group_check=True)

        def evac(pt, w, nm):
            t = sbT.tile([w, DT], BF16, tag=nm, name=f"{nm}_{td}")
            nc.scalar.copy(t[:], pt[0:w, :])
            return t

        # dtp1 paths
        p_ss = psp.tile([64, DT], F32, tag="psp", name=uid("pss"))
        path(W3nf, 0, 32, [s2rep], UBss, 64, lambda xi: (p_ss, 0))
        bil1 = evac(p_ss, 64, "bil1")
        p_vs = psp.tile([64, DT], F32, tag="psp", name=uid("pvs"))
        path(W3nf, 6144, 16, [s2rep], UBss, 64, lambda xi: (p_vs, 0))
        bvsR = sbT.tile([96, DT], BF16, tag="bvsR", name=f"bvsR_{td}")
        for x in range(3):
            nc.scalar.copy(bvsR[32 * x:32 * x + 32, :], p_vs[0:32, :])
        p_svA = psp.tile([128, DT], F32, tag="psp", name=uid("psa"))
        p_svB = psp.tile([64, DT], F32, tag="psp", name=uid("psb2"))
        path(W3nf, 4096, 16, v2rep, UBsv, 64,
             lambda xi: (p_svA, 64 * xi) if xi < 2 else (p_svB, 0))
        bsv = []
        for x in range(3):
            t = sbT.tile([64, DT], BF16, tag=f"bsv{x}", name=f"bsv{x}_{td}")
            nc.scalar.copy(t[:], p_svA[64 * x:64 * x + 64, :] if x < 2
                           else p_svB[0:64, :])
            bsv.append(t)
        p_v0 = psp.tile([96, DT], F32, tag="psp", name=uid("pv0"))
        path(W3nf, 8192, 8, v2rep, UB4, 32, lambda xi: (p_v0, 32 * xi))
        bv0 = evac(p_v0, 96, "bv0")
        p_v1 = psp.tile([96, DT], F32, tag="psp", name=uid("pv1"))
        path(W3nf, 9216, 8, v2rep, UB4, 32, lambda xi: (p_v1, 32 * xi))
        # cbuf in two section rotations: P1[s]=cbuf[(s+1)%3], P2[s]=cbuf[(s+2)%3]
        cbufP1 = sbT.tile([96, DT], BF16, tag="cbufP1", name=f"cbufP1_{td}")
        cbufP2 = sbT.tile([96, DT], BF16, tag="cbufP2", name=f"cbufP2_{td}")
        for s_ in range(3):
            nc.scalar.copy(cbufP1[32 * s_:32 * s_ + 32, :],
                           p_v1[32 * ((s_ + 1) % 3):32 * ((s_ + 1) % 3) + 32, :])
            nc.scalar.copy(cbufP2[32 * s_:32 * s_ + 32, :],
                           p_v1[32 * ((s_ + 2) % 3):32 * ((s_ + 2) % 3) + 32, :])

        # ---- epilogue 1: fsT [96, DT], fvT_x [128, DT] (bf16) ----
        fsT = sbT.tile([96, DT], BF16, tag="fsT", name=f"fsT_{td}")
        nc.vector.tensor_tensor(fsT[0:64, :], bil1[:], s1T[:], op=OP.mult)
        t96 = sbT.tile([96, DT], BF16, tag="t96", name=f"t96_{td}")
        nc.vector.tensor_tensor(t96[:], v1T[:], bv0[:], op=OP.mult)
        p_x = psm.tile([32, DT], F32, tag="ps_med", name=uid("px"))
        nc.tensor.matmul(p_x[:], U3[:], t96[:], start=True, stop=True)
        nc.scalar.copy(fsT[64:96, :], p_x[:])
        nc.vector.tensor_scalar(fsT[:], fsT[:], bcols[0:96, 0:1], None, op0=OP.add)

        # cross products, all base-aligned: ta[s] = v1T[s]*cbuf[(s+1)%3],
        # tb[s] = v1T[s]*cbuf[(s+2)%3]; cross_x = ta[(x+1)%3] - tb[(x+2)%3]
        ta_all = sbq.tile([96, DT], BF16, tag="ta", name=uid("ta"))
        nc.vector.tensor_tensor(ta_all[:], v1T[:], cbufP1[:], op=OP.mult)
        tb_all = sbq.tile([96, DT], BF16, tag="tb", name=uid("tb"))
        nc.vector.tensor_tensor(tb_all[:], v1T[:], cbufP2[:], op=OP.mult)
        # rotate tb by +1 section so cross_all[s] = ta_all[s] - tb_rot[s]
        # with cross_all[(x+1)%3] = cross_x
        tb_rot = sbq.tile([96, DT], BF16, tag="tbr", name=uid("tbr"))
        for s_ in range(3):
            nc.vector.tensor_copy(tb_rot[32 * s_:32 * s_ + 32, :],
                                  tb_all[32 * ((s_ + 1) % 3):32 * ((s_ + 1) % 3) + 32, :])
        cross_all = sbq.tile([96, DT], BF16, tag="cra", name=uid("cr"))
        nc.vector.tensor_sub(cross_all[:], ta_all[:], tb_rot[:])
        fvT = []
        for x in range(3):
            fv = sbT.tile([128, DT], BF16, tag=f"fvT{x}", name=f"fvT{x}_{td}")
            nc.vector.tensor_tensor(fv[0:64, :], bsv[x][:], s1T[:], op=OP.mult)
            nc.vector.tensor_tensor(fv[64:96, :], v1T[32 * x:32 * x + 32, :],
                                    bvsR[32 * x:32 * x + 32, :], op=OP.mult)
            yx = (x + 1) % 3
            nc.vector.tensor_copy(fv[96:128, :], cross_all[32 * yx:32 * yx + 32, :])
            fvT.append(fv)

        # ---- node-fusion linear (reads transposed directly) ----
        pns = psm.tile([64, DT], F32, tag="ps_med", name=uid("pns"))
        nc.tensor.matmul(pns[:], ntWs[:], fsT[:], start=True, stop=True)
        nsT = sbT.tile([64, DT], BF16, tag="nsT", name=f"nsT_{td}")
        nc.vector.tensor_scalar(nsT[:], pns[:], bcols[0:64, 2:3], None, op0=OP.add)
        pnv = psm.tile([96, DT], F32, tag="ps_med", name=uid("pnv"))
        for x in range(3):
            nc.tensor.matmul(pnv[32 * x:32 * x + 32, :], ntWv[:], fvT[x][:],
                             start=True, stop=True, skip_group_check=True)
        nvT = sbT.tile([96, DT], BF16, tag="nvT", name=f"nvT_{td}")
        nc.scalar.copy(nvT[:], pnv[:])

        # ---- dtp2 ----
        p_ss2 = psp.tile([64, DT], F32, tag="psp", name=uid("ps2"))
        path(W3ef, 0, 32, [esrep], UBss, 64, lambda xi: (p_ss2, 0))
        bil2 = evac(p_ss2, 64, "bil2")
        p_v02 = psp.tile([96, DT], F32, tag="psp", name=uid("pv2"))
        path(W3ef, 4096, 8, evrep, UB4, 32, lambda xi: (p_v02, 32 * xi))
        bv02 = evac(p_v02, 96, "bv02")

        asT = sbT.tile([96, DT], BF16, tag="asT", name=f"asT_{td}")
        nc.vector.tensor_tensor(asT[0:64, :], bil2[:], nsT[:], op=OP.mult)
        t96b = sbT.tile([96, DT], BF16, tag="t96b", name=f"t96b_{td}")
        nc.vector.tensor_tensor(t96b[:], nvT[:], bv02[:], op=OP.mult)
        p_x2 = psm.tile([32, DT], F32, tag="ps_med", name=uid("px2"))
        nc.tensor.matmul(p_x2[:], U3[:], t96b[:], start=True, stop=True)
        nc.scalar.copy(asT[64:96, :], p_x2[:])
        nc.vector.tensor_scalar(asT[:], asT[:], bcols[0:96, 1:2], None, op0=OP.add)

        # defer adaLN/head/scatter: collect per-512 state
        asTs.append(asT)
        g_modA += g_mod; d_colsA += d_cols; rvsA += rvs; srcfsA += srcfs

    # ---- deferred tail, stage-major across all 8 subtiles ----
    NS8 = NDT * NSUB
    as_l = []
    for k in range(NS8):
        td, s = divmod(k, NSUB)
        as_p = psb.tile([P, S_TP], BF16, tag="ps_bf", name=uid("asp"))
        nc.tensor.transpose(as_p[:], asTs[td][:, s * P:s * P + P],
                            identB[0:96, 0:96])
        as_ = sbc.tile([P, S_TP], F32, tag="as", name=uid("as"))
        nc.scalar.copy(as_[:], as_p[:])
        as_l.append(as_)
    mus = [sbc.tile([P, 1], F32, tag="amu", name=uid("am")) for _ in range(NS8)]
    for k in range(NS8):
        nc.vector.tensor_reduce(mus[k][:], as_l[k][:], axis=AX.X, op=OP.add)
    for k in range(NS8):
        nc.vector.tensor_scalar_mul(mus[k][:], mus[k][:], 1.0 / S_TP)
    cens = [sbc.tile([P, S_TP], F32, tag="acen", name=uid("ac"))
            for _ in range(NS8)]
    for k in range(NS8):
        nc.vector.tensor_scalar(cens[k][:], as_l[k][:], mus[k][:, :1], None,
                                op0=OP.subtract)
    vars_ = [sbc.tile([P, 1], F32, tag="avar", name=uid("av"))
             for _ in range(NS8)]
    for k in range(NS8):
        sqv = sb.tile([P, S_TP], F32, tag="asq", name=uid("aq"))
        nc.scalar.activation(sqv[:], cens[k][:], ACTF.Square,
                             accum_out=vars_[k][:])
    rstds = []
    for k in range(NS8):
        rstds.append(rstd_from_var(vars_[k][:], 1.0 / S_TP, sbc))
    s_ns = [sbc.tile([P, S_TP], BF16, tag="sn", name=uid("sn"))
            for _ in range(NS8)]
    for k in range(NS8):
        nc.vector.scalar_tensor_tensor(s_ns[k][:], cens[k][:],
                                       rstds[k][:, :1],
                                       g_modA[k][:, S_TP:2 * S_TP],
                                       op0=OP.mult, op1=OP.mult)
    for k in range(NS8):
        nc.vector.tensor_tensor(s_ns[k][:], s_ns[k][:], g_modA[k][:, 0:S_TP],
                                op=OP.add)
    hds = []
    for k in range(NS8):
        snT_p = psb.tile([S_TP, P], BF16, tag="ps_bf", name=uid("snp"))
        nc.tensor.transpose(snT_p[:], s_ns[k][:], identB[:])
        snT = sbc.tile([S_TP, P], BF16, tag="snT", name=uid("snT"))
        nc.scalar.copy(snT[:], snT_p[:])
        hd_p = ps.tile([P, 32], F32, tag="ps_small", name=uid("hdp"))
        nc.tensor.matmul(hd_p[:], snT[:], spW1[:], start=True, stop=True)
        hd = sbc.tile([P, 32], F32, tag="hd", name=uid("hd"))
        nc.vector.tensor_tensor(hd[:], hd_p[:], rows_slice(rep, 'sp_b1'),
                                op=OP.add)
        hds.append(hd)
    es_ = [sbc.tile([P, 32], F32, tag="hse", name=uid("he")) for _ in range(NS8)]
    for k in range(NS8):
        nc.scalar.activation(es_[k][:], hds[k][:], ACTF.Exp, scale=-1.0)
    for k in range(NS8):
        nc.vector.tensor_scalar_add(es_[k][:], es_[k][:], 1.0)
    rs_ = [sbc.tile([P, 32], F32, tag="hsr", name=uid("hr")) for _ in range(NS8)]
    for k in range(NS8):
        nc.vector.reciprocal(rs_[k][:], es_[k][:])
    siles = [sbc.tile([P, 32], F32, tag="sile", name=uid("sl"))
             for _ in range(NS8)]
    for k in range(NS8):
        nc.vector.tensor_tensor(siles[k][:], hds[k][:], rs_[k][:], op=OP.mult)
    sws = [sbc.tile([P, 1], F32, tag="swv", name=uid("sv")) for _ in range(NS8)]
    for k in range(NS8):
        swt = sb.tile([P, 32], F32, tag="swt", name=uid("sw"))
        nc.vector.tensor_tensor(swt[:], siles[k][:], rows_slice(rep, 'spW2r'),
                                op=OP.mult)
        nc.vector.tensor_reduce(sws[k][:], swt[:], axis=AX.X, op=OP.add)
        nc.vector.tensor_scalar(sws[k][:], sws[k][:], 32.0 ** -0.5,
                                rep[:, ROWS['sp_b2'][0]:ROWS['sp_b2'][0] + 1],
                                op0=OP.mult, op1=OP.add)
    forces = []
    for k in range(NS8):
        den = sbc.tile([P, 1], F32, tag="den", name=uid("dn"))
        nc.vector.scalar_tensor_tensor(den[:], d_colsA[k], 1.0, d_colsA[k],
                                       op0=OP.add, op1=OP.mult)
        rden = sbc.tile([P, 1], F32, tag="rden", name=uid("rd"))
        nc.vector.reciprocal(rden[:], den[:])
        coef = sbc.tile([P, 1], F32, tag="coef", name=uid("cf"))
        nc.vector.tensor_mul(coef[:], sws[k][:], rden[:])
        force = sbc.tile([P, 3], BF16, tag="force", name=uid("fo"))
        nc.vector.tensor_scalar(force[:], rvsA[k], coef[:, :1], None,
                                op0=OP.mult)
        forces.append(force)

    # scatter: one PSUM accumulator over all 8 subtiles
    acc_p = psp.tile([P, NCHUNK * 3], F32, tag="psp", name=uid("ap"))
    for ch in range(NCHUNK):
        ohs = []
        for k in range(NS8):
            ssh = sbc.tile([P, 1], F32, tag="ssh", name=uid("sh"))
            nc.vector.tensor_scalar_add(ssh[:], srcfsA[k], float(-P * ch))
            oh = sbc.tile([P, P], BF16, tag="oh", name=uid("oh"))
            nc.gpsimd.tensor_scalar(oh[:], iota_f[:], ssh[:, :1], None,
                                    op0=OP.is_equal)
            ohs.append(oh)
        for k in range(NS8):
            nc.tensor.matmul(acc_p[:, 3 * ch:3 * ch + 3], ohs[k][:],
                             forces[k][:],
                             start=(k == 0), stop=(k == NS8 - 1),
                             skip_group_check=True)
    acc_sb = consts.tile([P, NCHUNK * 3], F32)
    nc.scalar.copy(acc_sb[:], acc_p[:])
    dma(T['out'][:], acc_sb[:])


# ======================= host side =======================

def _bf16(a):
    return np.asarray(a, np.float32).astype(mybir.dt.np(BF16))


def host_prep(inp):
    inp = {k: np.asarray(v) for k, v in inp.items()}
    src = inp['edge_index'][0].astype(np.int32)
    dst = inp['edge_index'][1].astype(np.int32)
    perm = np.argsort(src, kind='stable')
    src, dst = src[perm], dst[perm]
    gid = inp['batch'].astype(np.int32)[src]
    h_edge = inp['h_edge'][perm]
    dist = inp['distance'][perm].astype(np.float32)
    rvec = inp['relative_vec'][perm].astype(np.float32)

    rows = np.zeros(RWID, np.float32)

    def setr(name, val):
        off, w = ROWS[name]
        rows[off:off + w] = val
    for p in ('nf', 'ef'):
        for q in ('g1', 'b1', 'g2', 'b2'):
            setr(f'{p}_{q}', inp[f'{p}_{q}'])
    setr('src_bs', inp['src_bs']); setr('dst_bs', inp['dst_bs'])
    setr('sp_b1', inp['sp_b1']); setr('spW2r', inp['sp_W2'][:, 0])
    rows[ROWS['sp_b2'][0]] = inp['sp_b2'][0]
    rows[ROWS['eps'][0]] = 1e-5
    setr('normbt', inp['norm_bt'][:2 * S_TP])

    std = np.asarray(inp['rbf_std'], np.float32)
    mean = np.asarray(inp['rbf_mean'], np.float32)
    rbf_w = float(np.asarray(inp['rbf_w'])); rbf_b = float(np.asarray(inp['rbf_b']))
    A = (rbf_w / (CUTOFF * std)).reshape(-1, 1).astype(np.float32)
    Bc = ((rbf_b - mean) / std).reshape(-1, 1).astype(np.float32)
    cnorm = (1.0 / (np.sqrt(2 * np.pi) * std)).reshape(-1, 1).astype(np.float32)
    W1p = np.concatenate([inp['nf_W1'], inp['ef_W1']], axis=1) * cnorm

    def fold_w3(W3):
        W3 = np.asarray(W3, np.float32).copy()
        W3[:, 0:4096] *= 64.0 ** -0.5
        W3[:, 4096:6144] *= 32.0 ** -0.5
        W3[:, 6144:8192] *= 64.0 ** -0.5
        W3[:, 8192:9216] *= 96.0 ** -0.5
        W3[:, 9216:10240] *= 64.0 ** -0.5
        return W3
    W3nf = fold_w3(inp['nf_W3'])
    W3ef_f = fold_w3(inp['ef_W3'])
    W3ef = np.concatenate([W3ef_f[:, :4096], W3ef_f[:, 8192:9216]], axis=1)

    Wsd = np.concatenate([inp['src_Ws'], inp['dst_Ws']], axis=1) * 128.0 ** -0.5
    Wvsd = np.concatenate([inp['src_Wv'], inp['dst_Wv']], axis=1) * 64.0 ** -0.5

    pp = np.arange(128)
    # UBss: block b (of 32) -> row 2b + p//64, packed as cols [64b, 64b+64)
    UBss = np.zeros((128, 2048), np.float32)
    for b in range(32):
        UBss[pp, 64 * b + np.minimum(2 * b + pp // 64, 63)] = (2 * b + pp // 64 < 64)
    # UBsv: block b (of 16) -> row 4b + p//32, cols [64b, 64b+64)
    UBsv = np.zeros((128, 1024), np.float32)
    for b in range(16):
        UBsv[pp, 64 * b + 4 * b + pp // 32] = 1.0
    # UB4: block b (of 8) -> row 4b + p//32, cols [32b, 32b+32)
    UB4 = np.zeros((128, 256), np.float32)
    for b in range(8):
        UB4[pp, 32 * b + 4 * b + pp // 32] = 1.0
    U3 = (np.arange(96)[:, None] % 32 == np.arange(32)[None, :]).astype(np.float32)
    R96 = np.zeros((96, 384), np.float32)
    for x in range(3):
        R96[:, 128 * x:128 * (x + 1)] = (
            np.arange(96)[:, None] == 32 * x + np.arange(128)[None, :] % 32)
    bcols = np.zeros((96, 4), np.float32)
    bcols[:, 0] = inp['nf_bias']; bcols[:, 1] = inp['ef_bias']
    bcols[:64, 2] = inp['nt_bs']; bcols[:64, 3] = inp['et_bs']

    # device row order: scalars then x-grouped vector components
    hnT = np.ascontiguousarray(inp['h_node'].T)          # [320, N]
    hnT = np.concatenate([hnT[0:128], hnT[128::3], hnT[129::3], hnT[130::3]], 0)
    shared = dict(
        hn_T=_bf16(hnT),
        t_T=_bf16(np.ascontiguousarray(inp['t'].T)),
        normWt=_bf16(np.ascontiguousarray(inp['norm_Wt'][:, :2 * S_TP])),
        rows=np.ascontiguousarray(rows.reshape(1, -1), np.float32),
        rbf_A=A, rbf_B=Bc,
        W1p=_bf16(W1p),
        nf_W2=_bf16(inp['nf_W2']), ef_W2=_bf16(inp['ef_W2']),
        W3nf=_bf16(W3nf), W3ef=_bf16(W3ef),
        Wsd=_bf16(Wsd), Wvsd=_bf16(Wvsd),
        nt_Ws=_bf16(inp['nt_Ws'] * 96.0 ** -0.5),
        nt_Wv=_bf16(inp['nt_Wv'] * 128.0 ** -0.5),
        et_Ws=_bf16(inp['et_Ws'] * 64.0 ** -0.5),
        et_Wv=_bf16(inp['et_Wv'] * 32.0 ** -0.5),
        sp_W1=_bf16(inp['sp_W1'] * 96.0 ** -0.5),
        UBss=_bf16(UBss), UBsv=_bf16(UBsv), UB4=_bf16(UB4),
        U3=_bf16(U3), R96=_bf16(R96),
        bcols=bcols,
    )

    in_maps = []
    for c in range(NC_CORES):
        sl = slice(c * EC, (c + 1) * EC)
        m = dict(shared)
        heT_ = np.ascontiguousarray(h_edge[sl].T)        # [160, EC]
        heT_ = np.concatenate([heT_[0:64], heT_[64::3], heT_[65::3], heT_[66::3]], 0)
        m['he_T'] = _bf16(heT_)
        m['dist_r'] = np.ascontiguousarray(dist[sl].reshape(1, -1))
        m['edgef'] = np.ascontiguousarray(np.concatenate(
            [dist[sl, None], rvec[sl], src[sl, None].astype(np.float32)],
            axis=1), np.float32)
        m['edgei'] = np.ascontiguousarray(np.stack(
            [src[sl], dst[sl], gid[sl]], axis=1).astype(np.int32))
        in_maps.append(m)
    return in_maps


_CACHED_NC = None


def kernel(**inputs):
    global _CACHED_NC
    from concourse.bass_utils import run_bass_kernel_spmd
    if _CACHED_NC is None:
        _CACHED_NC = build_nc()
    in_maps = host_prep(inputs)
    res = run_bass_kernel_spmd(_CACHED_NC, in_maps, list(range(NC_CORES)))
    out = np.zeros((128, NCHUNK, 3), np.float32)
    for r in res.results:
        out += r['out'].reshape(128, NCHUNK, 3)
    return np.ascontiguousarray(out.transpose(1, 0, 2).reshape(N, 3))
